# revision 35
# baseline (speedup 1.0000x reference)
"""Trainium2 Bass kernel for the ELGCA block (dwconv3x3+gelu || conv1x1+gelu
-> pooled linear attention), data-parallel over batch on 8 NeuronCores.

Self-contained: hardcodes shapes B=16, C=128, H=W=128, f32 I/O.
kernel(**inputs) takes full unsharded inputs, returns the FULL f32 output.

v8 (per core, BPC=2 local images, partitions p = b*64 + c):
  - fp16 end-to-end on the matmul paths: the host pre-converts x to fp16
    (halves HBM read traffic, removes all on-chip casts); every PE matmul
    runs at 1 cycle/row.  A-side (q|k) precision in fp16 gives ~1.2e-2
    rel err end-to-end (bf16 fails at 0.34: softmax-logit amplification
    needs >=10 mantissa bits).  Pooling / qk / softmax stay f32.
  - dwconv3x3: 7 taps on PE as diagonal fp16 matmuls (tap-major LDW),
    2 taps (dx=1) on DVE in 4x/2x perf mode over the flat padded slab,
    merged via an identity matmul.  dwconv PSUM is half-window double
    buffered so next window's taps never wait on this window's gelu.
  - PE kept at HAM K=8/8: warm-up matmuls at start + through the serial
    softmax stretch; per-window emission order keeps the PE queue fed.
  - DMA: bulk stores (x1, out2) + slab loads on gpsimd SWDGE (spreads
    across all 16 SDMA engines; HWDGE rings degrade to ~2 engines for
    stores), x2 loads on scalar HWDGE, l stores on sync HWDGE.
  - outputs written fp16, widened to f32 on the host.
"""

import numpy as np
from contextlib import ExitStack

import concourse.bass as bass
import concourse.tile as tile
from concourse import bacc, mybir
from concourse import bass_utils
from concourse.masks import make_identity

F32 = mybir.dt.float32
FP16 = mybir.dt.float16
AX = mybir.AxisListType
ALU = mybir.AluOpType
ACTF = mybir.ActivationFunctionType

N_CORES = 8
B_TOT, C, H, W = 16, 128, 128, 128
BPC = B_TOT // N_CORES          # 2 images per core
HW = H * W                      # 16384
C2 = C // 2                     # 64
C4 = C // 4                     # 32
WP = W + 2                      # padded row width (130)
NW = 8                          # number of 16-row windows
WR = H // NW                    # image rows per window (16)
NP = (H // 2) * (W // 2)        # 4096 pooled positions
W2 = W // 2                     # 64
FLAT = 18 * WP                  # flat padded slab size (2340)

# taps: index t = dy*3+dx; PE takes 7, DVE takes 2 (dx=1 keeps the flat
# contiguous offset 4B-aligned for the DVE 2x/4x perf modes)
PE_TAPS = [(0, 0), (0, 1), (0, 2), (1, 0), (1, 2), (2, 0), (2, 2)]


def build_nc(loops=1):
    nc = bacc.Bacc("TRN2", target_bir_lowering=False, debug=False,
                   num_devices=N_CORES)
    x = nc.dram_tensor("x", [BPC, C, H, W], FP16, kind="ExternalInput").ap()
    dw_w = nc.dram_tensor("dw_w", [C2, 1, 3, 3], F32, kind="ExternalInput").ap()
    dw_b = nc.dram_tensor("dw_b", [C2], F32, kind="ExternalInput").ap()
    qw = nc.dram_tensor("qkvl_w", [C, C2, 1, 1], F32, kind="ExternalInput").ap()
    qb = nc.dram_tensor("qkvl_b", [C], F32, kind="ExternalInput").ap()
    out = nc.dram_tensor("out", [BPC, C, H, W], FP16, kind="ExternalOutput").ap()

    with tile.TileContext(nc) as tc, ExitStack() as ctx:
        consts = ctx.enter_context(tc.tile_pool(name="consts", bufs=1))
        inp = ctx.enter_context(tc.tile_pool(name="inp", bufs=4))
        bigp = ctx.enter_context(tc.tile_pool(name="bigp", bufs=1))
        stgp = ctx.enter_context(tc.tile_pool(name="stgp", bufs=2))
        ps = ctx.enter_context(tc.tile_pool(name="ps", bufs=1, space="PSUM"))

        id_f32 = consts.tile([128, 128], F32)
        make_identity(nc, id_f32[:])

        def issue_inputs(w):
            """DMA window w's inputs and return (slab, slab3, x2t)."""
            y0 = w * WR
            ys = max(y0 - 1, 0)
            ye = min(y0 + WR + 1, H)
            nrows = ye - ys
            rs = 0 if w > 0 else 1
            slab = inp.tile([128, FLAT], FP16, tag="slab")
            slab3 = slab.rearrange("p (r w) -> p r w", w=WP)
            nc.vector.memset(slab3[:, :, 0:1], 0.0)
            nc.vector.memset(slab3[:, :, WP - 1:WP], 0.0)
            if w == 0:
                nc.vector.memset(slab3[:, 0:1, :], 0.0)
            if w == NW - 1:
                nc.vector.memset(slab3[:, 17:18, :], 0.0)
            for b in range(BPC):
                nc.gpsimd.dma_start(
                    slab3[C2 * b:C2 * b + C2, rs:rs + nrows, 1:W + 1],
                    x[b:b + 1, 0:C2, ys:ye, :])
            x2t = inp.tile([128, WR * W], FP16, tag="x2t")
            x2t3 = x2t.rearrange("p (r w) -> p r w", w=W)
            nc.gpsimd.dma_start(x2t3[:, :, :],
                                x[0:BPC, C2:C, y0:y0 + WR, :])
            return slab, slab3, x2t

        # input DMAs for the first windows go out before the consts
        # chain occupies the queues
        pend = [issue_inputs(0), issue_inputs(1), issue_inputs(2)]

        # warm the PE (HAM throttle) while the first slab DMA is in
        # flight: dead accumulating matmuls on the identity
        wup = ps.tile([128, 1024], F32, tag="cv", bufs=2)
        for i in range(20):
            nc.tensor.matmul(wup[:, 0:128], id_f32[:], id_f32[:],
                             start=(i == 0), stop=(i == 19))

        # ---------------- constants (sync-queue loads) ----------------
        w_tile = consts.tile([128, 9], F32)
        dw9 = dw_w.rearrange("c o kh kw -> c (o kh kw)")
        nc.sync.dma_start(w_tile[0:C2, :], dw9)
        nc.sync.dma_start(w_tile[C2:128, :], dw9)

        # qkvl_w: load [128oc, 64ic] contiguous, PE-transpose to [64ic, 128oc]
        qw_oc = consts.tile([128, C2], F32)
        nc.sync.dma_start(qw_oc[:], qw.rearrange("o i kh kw -> o (i kh kw)"))
        qwT_ps = ps.tile([128, 1024], F32, tag="cv", bufs=2)
        nc.tensor.transpose(qwT_ps[0:C2, 0:128], qw_oc[:], id_f32[:])
        qwT_sb = consts.tile([C2, 128], F32)
        nc.scalar.copy(qwT_sb[:], qwT_ps[0:C2, 0:128])

        # row-vector loads (1 descriptor each) for biases
        qb_row = consts.tile([1, C], F32)
        nc.sync.dma_start(qb_row[:], qb.unsqueeze(0))
        dwb_row = consts.tile([1, C2], F32)
        nc.sync.dma_start(dwb_row[:], dw_b.unsqueeze(0))

        # permuted bias rows -> PE transpose to per-partition columns
        # brow col-blocks: biasA = [qb0:32|qb0:32|qb32:64|qb32:64],
        # biasB = [qb64:96|...], dwb = [dwb|dwb]
        brow = consts.tile([1, 3 * 128], F32)
        nc.scalar.copy(brow[:, 0:C4], qb_row[:, 0:C4])
        nc.scalar.copy(brow[:, C4:C2], qb_row[:, 0:C4])
        nc.scalar.copy(brow[:, C2:96], qb_row[:, C4:C2])
        nc.scalar.copy(brow[:, 96:128], qb_row[:, C4:C2])
        nc.scalar.copy(brow[:, 128:160], qb_row[:, C2:96])
        nc.scalar.copy(brow[:, 160:192], qb_row[:, C2:96])
        nc.scalar.copy(brow[:, 192:224], qb_row[:, 96:128])
        nc.scalar.copy(brow[:, 224:256], qb_row[:, 96:128])
        nc.scalar.copy(brow[:, 256:320], dwb_row[:, 0:C2])
        nc.scalar.copy(brow[:, 320:384], dwb_row[:, 0:C2])
        bcol_ps = ps.tile([128, 1024], F32, tag="dw", bufs=2)
        for i in range(3):
            nc.tensor.transpose(bcol_ps[:, i:i + 1],
                                brow[0:1, i * 128:(i + 1) * 128],
                                id_f32[0:1, 0:1])
        bcol = consts.tile([128, 3], F32)
        nc.scalar.copy(bcol[:], bcol_ps[:, 0:3])
        biasA = bcol[:, 0:1]
        biasB = bcol[:, 1:2]
        dwb_t = bcol[:, 2:3]

        # 9 diagonal tap matrices, fp16 (direct f32->fp16 on DVE)
        wdiag = consts.tile([128, 9 * 128], FP16)
        for t in range(9):
            nc.vector.tensor_scalar_mul(
                wdiag[:, t * 128:(t + 1) * 128], id_f32[:],
                w_tile[:, t:t + 1])

        # conv1x1 block-diagonal weights from qwT_sb (direct fp16 copies)
        lhsAB = consts.tile([128, 256], FP16)
        nc.vector.memset(lhsAB[:], 0.0)
        nc.scalar.copy(lhsAB[0:C2, 0:C4], qwT_sb[:, 0:C4])
        nc.scalar.copy(lhsAB[C2:128, C4:C2], qwT_sb[:, 0:C4])
        nc.scalar.copy(lhsAB[0:C2, C2:96], qwT_sb[:, C4:C2])
        nc.scalar.copy(lhsAB[C2:128, 96:128], qwT_sb[:, C4:C2])
        nc.scalar.copy(lhsAB[0:C2, 128:160], qwT_sb[:, C2:96])
        nc.scalar.copy(lhsAB[C2:128, 160:192], qwT_sb[:, C2:96])
        nc.scalar.copy(lhsAB[0:C2, 192:224], qwT_sb[:, 96:128])
        nc.scalar.copy(lhsAB[C2:128, 224:256], qwT_sb[:, 96:128])
        id_h = consts.tile([128, 128], FP16)
        nc.vector.tensor_copy(id_h[:], id_f32[:])
        lhsA = lhsAB[:, 0:128]
        lhsB = lhsAB[:, 128:256]

        def one_pass():
            # persistent per-pass buffers
            vl = bigp.tile([128, HW], FP16, tag="vl")   # v rows 0:64, l 64:128
            hp = bigp.tile([128, H * W2], F32, tag="hp")
            hp3 = hp.rearrange("p (r w) -> p r w", w=W2)
            hpv = hp.rearrange("p (o two w) -> p o two w", two=2, w=W2)
            pp = bigp.tile([128, NP], F32, tag="pp")
            pp3 = pp.rearrange("p (r w) -> p r w", w=W2)
            qk_acc = bigp.tile([C2, C2], F32, tag="qk")

            def attn_pp(sw):
                """v-pool pooled rows [8sw, 8sw+8) (DVE part of attn)."""
                o0 = 8 * sw
                nc.vector.tensor_add(pp3[0:C2, o0:o0 + 8, :],
                                     hpv[0:C2, o0:o0 + 8, 0, :],
                                     hpv[0:C2, o0:o0 + 8, 1, :])
                lo = max(o0, 1)
                nc.vector.tensor_add(pp3[0:C2, lo:o0 + 8, :],
                                     pp3[0:C2, lo:o0 + 8, :],
                                     hpv[0:C2, lo - 1:o0 + 7, 1, :])
                nc.vector.tensor_max(pp3[C2:128, o0:o0 + 8, :],
                                     hpv[C2:128, o0:o0 + 8, 0, :],
                                     hpv[C2:128, o0:o0 + 8, 1, :])

            def attn_tr(sw):
                """transpose the 4 new 128-position chunks."""
                trps = ps.tile([128, 1024], F32, tag="cv", bufs=2)
                for jj in range(4):
                    ch = 4 * sw + jj
                    nc.tensor.transpose(trps[:, jj * 128:(jj + 1) * 128],
                                        pp[:, ch * 128:(ch + 1) * 128],
                                        id_f32[:])
                trsb = stgp.tile([128, 512], FP16, tag="trsb")
                nc.scalar.copy(trsb[:], trps[:, 0:512])
                return trsb

            def attn_qk(sw, trsb):
                """accumulate this slice's qk partial."""
                qkps = ps.tile([128, 1024], F32, tag="cv", bufs=2)
                for jj in range(4):
                    nc.tensor.matmul(
                        qkps[0:C2, 0:C2],
                        trsb[:, jj * 128 + C2:(jj + 1) * 128],
                        trsb[:, jj * 128:jj * 128 + C2],
                        start=(jj == 0), stop=(jj == 3))
                if sw == 0:
                    nc.scalar.copy(qk_acc[:], qkps[0:C2, 0:C2])
                else:
                    qktmp = stgp.tile([C2, C2], F32, tag="qktmp")
                    nc.vector.tensor_copy(qktmp[:], qkps[0:C2, 0:C2])
                    nc.gpsimd.tensor_add(qk_acc[:], qk_acc[:], qktmp[:])

            for w in range(NW):
                y0 = w * WR
                slab, slab3, x2t = pend.pop(0)
                if w + 3 < NW:
                    pend.append(issue_inputs(w + 3))

                qg = stgp.tile([128, WR * W], F32, tag="qg", bufs=1)
                x1st = stgp.tile([128, WR * W], FP16, tag="x1st")

                # ---- PE dwconv taps first: keeps the PE streaming while
                # DVE computes the dx=1 taps and the attn pooling.
                # Tap-major across both halves: one LDWEIGHTS per tap. ----
                dwps = []
                for _h in range(2):
                    dwtile = ps.tile([128, 1024], F32, tag="dw", bufs=2)
                    dwps.append(dwtile)
                for ti, (dy, dx) in enumerate(PE_TAPS):
                    t = dy * 3 + dx
                    for h in range(2):
                        for qq in range(2):
                            la = h * 8 + qq * 4
                            nc.tensor.matmul(
                                dwps[h][:, qq * 512:(qq + 1) * 512],
                                wdiag[:, t * 128:(t + 1) * 128],
                                slab3[:, la + dy:la + dy + 4, dx:dx + W],
                                start=(ti == 0), stop=False)

                # ---- attn pooling + transposes for the previous window ----
                trsb = None
                if w >= 1:
                    attn_pp(w - 1)
                    trsb = attn_tr(w - 1)

                # ---- dx=1 dwconv taps on DVE (flat contiguous, 4x/2x) ----
                dwacc = stgp.tile([128, 16 * WP], FP16, tag="dwacc", bufs=1)
                dwtmp = stgp.tile([128, 16 * WP], FP16, tag="dwtmp", bufs=1)
                nc.vector.tensor_scalar_mul(
                    dwacc[:, 0:16 * WP], slab[:, WP:WP + 16 * WP],
                    w_tile[:, 4:5])
                nc.vector.tensor_scalar_mul(
                    dwtmp[:, 0:16 * WP], slab[:, 2 * WP:2 * WP + 16 * WP],
                    w_tile[:, 7:8])
                nc.vector.tensor_add(dwacc[:, 0:16 * WP],
                                     dwacc[:, 0:16 * WP],
                                     dwtmp[:, 0:16 * WP])
                dwacc3 = dwacc.rearrange("p (r w) -> p r w", w=WP)

                # ---- conv1x1 A pairs (fp16 -> f32 qg) ----
                for pr in range(2):
                    pc0 = pr * 1024
                    Aps = ps.tile([128, 1024], F32, tag="cv", bufs=2)
                    for hf in range(2):
                        nc.tensor.matmul(
                            Aps[:, hf * 512:(hf + 1) * 512], lhsA,
                            x2t[:, pc0 + hf * 512:pc0 + (hf + 1) * 512],
                            start=True, stop=True)
                    nc.scalar.activation(qg[:, pc0:pc0 + 1024], Aps[:],
                                         ACTF.Gelu, bias=biasA)

                # ---- attn qk for the previous window ----
                if w >= 1:
                    attn_qk(w - 1, trsb)

                # ---- conv1x1 B pairs -> vl ----
                for pr in range(2):
                    pc0 = pr * 1024
                    Bps = ps.tile([128, 1024], F32, tag="cv", bufs=2)
                    for hf in range(2):
                        nc.tensor.matmul(
                            Bps[:, hf * 512:(hf + 1) * 512], lhsB,
                            x2t[:, pc0 + hf * 512:pc0 + (hf + 1) * 512],
                            start=True, stop=True)
                    nc.scalar.activation(vl[:, y0 * W + pc0:
                                            y0 * W + pc0 + 1024],
                                         Bps[:], ACTF.Gelu, bias=biasB)

                # ---- merge DVE taps into the PE accumulators, gelu ----
                for h in range(2):
                    dw = dwps[h]
                    for qq in range(2):
                        la = h * 8 + qq * 4
                        nc.tensor.matmul(
                            dw[:, qq * 512:(qq + 1) * 512], id_h[:],
                            dwacc3[:, la:la + 4, 1:W + 1],
                            start=False, stop=True)
                    nc.scalar.activation(x1st[:, h * 1024:(h + 1) * 1024],
                                         dw[:], ACTF.Gelu, bias=dwb_t)
                x1st3 = x1st.rearrange("p (r w) -> p r w", w=W)
                nc.scalar.dma_start(out[0:BPC, 0:C2, y0:y0 + WR, :],
                                    x1st3[:, :, :])

                # ---- l output DMA for this window ----
                nc.gpsimd.dma_start(
                    out[0:BPC, C2:96, y0:y0 + WR, :],
                    vl[C2:128, y0 * W:(y0 + WR) * W]
                    .rearrange("p (r w) -> p r w", w=W))

                # ---- horizontal pooling for this window ----
                qg3 = qg.rearrange("p (r w2 two) -> p r w2 two", two=2, w2=W2)
                nc.vector.tensor_add(hp3[0:C2, y0:y0 + WR, :],
                                     qg3[0:C2, :, :, 0], qg3[0:C2, :, :, 1])
                nc.vector.tensor_add(hp3[0:C2, y0:y0 + WR, 1:W2],
                                     hp3[0:C2, y0:y0 + WR, 1:W2],
                                     qg3[0:C2, :, 0:W2 - 1, 1])
                nc.vector.tensor_max(hp3[C2:128, y0:y0 + WR, :],
                                     qg3[C2:128, :, :, 0],
                                     qg3[C2:128, :, :, 1])

            attn_pp(NW - 1)
            attn_qk(NW - 1, attn_tr(NW - 1))

            # keep the PE (HAM) warm through the serial softmax stretch:
            # dead accumulating matmuls, emitted BEFORE the Ebd transpose
            # so they sit ahead of it in the PE's in-order queue
            warmps = ps.tile([128, 1024], F32, tag="cv", bufs=2)
            for i in range(24):
                nc.tensor.matmul(warmps[:, 0:512], id_h[:],
                                 vl[:, i * 512:(i + 1) * 512],
                                 start=(i == 0), stop=(i == 23))

            # ---------- softmax stats -> block-diag attention ----------
            # Both image blocks in one pass: mask off-diagonal blocks to
            # -inf so the row reduce/exp/sum ignore them (exp -> 0, which
            # also zeroes the off-blocks of Ebd for free).
            Ebd = bigp.tile([C2, C2], FP16, tag="Ebd")
            qk9 = bigp.tile([C2, C2], F32, tag="qk9")
            nc.scalar.mul(qk9[:], qk_acc[:], 1.0 / 9.0)
            nc.vector.memset(qk9[0:C4, C4:C2], -1e30)
            nc.vector.memset(qk9[C4:C2, 0:C4], -1e30)
            nmax = bigp.tile([C2, 1], F32, tag="nmax")
            nc.vector.tensor_reduce(nmax[:], qk9[:], axis=AX.X,
                                    op=ALU.max, negate=True)
            ET = bigp.tile([C2, C2], F32, tag="ET")
            nc.scalar.activation(ET[:], qk9[:], ACTF.Exp,
                                 bias=nmax[:, 0:1])
            ssum = bigp.tile([C2, 1], F32, tag="ssum")
            nc.vector.reduce_sum(ssum[:], ET[:], axis=AX.X)
            rec = bigp.tile([C2, 1], F32, tag="rec")
            nc.vector.reciprocal(rec[:], ssum[:])
            ETn = bigp.tile([C2, C2], F32, tag="ETn")
            nc.vector.tensor_scalar_mul(ETn[:], ET[:], rec[:, 0:1])
            etp = ps.tile([128, 1024], F32, tag="dw", bufs=2)
            nc.tensor.transpose(etp[0:C2, 0:C2], ETn[:],
                                id_f32[0:C2, 0:C2])
            nc.scalar.copy(Ebd[:], etp[0:C2, 0:C2])

            # ---------- out2 = attn @ v, both batches per matmul ----------
            for w in range(NW):
                y0 = w * WR
                o2st = stgp.tile([C2, WR * W], FP16, tag="o2st", bufs=8)
                for pr in range(2):
                    pc0 = pr * 1024
                    o2ps = ps.tile([128, 1024], F32,
                                   tag=("cv" if pr == 0 else "dw"), bufs=2)
                    for hf in range(2):
                        nc.tensor.matmul(
                            o2ps[0:C2, hf * 512:(hf + 1) * 512], Ebd[:],
                            vl[0:C2, y0 * W + pc0 + hf * 512:
                               y0 * W + pc0 + (hf + 1) * 512],
                            start=True, stop=True)
                    if pr == 0:
                        nc.scalar.copy(o2st[:, pc0:pc0 + 1024],
                                       o2ps[0:C2, :])
                    else:
                        nc.vector.tensor_copy(o2st[:, pc0:pc0 + 1024],
                                              o2ps[0:C2, :])
                o2st3 = o2st.rearrange("p (r w) -> p r w", w=W)
                nc.gpsimd.dma_start(
                    out[0:BPC, 96:128, y0:y0 + WR, :],
                    o2st3[:, :, :])

        for _ in range(loops):
            one_pass()

    nc.compile()
    return nc


_NC_CACHE = None


def _get_nc():
    global _NC_CACHE
    if _NC_CACHE is None:
        _NC_CACHE = build_nc()
    return _NC_CACHE


def kernel(x, dw_w, dw_b, qkvl_w, qkvl_b):
    x = np.ascontiguousarray(np.asarray(x).astype(np.float16))
    shared = {
        "dw_w": np.ascontiguousarray(np.asarray(dw_w, dtype=np.float32)),
        "dw_b": np.ascontiguousarray(np.asarray(dw_b, dtype=np.float32)),
        "qkvl_w": np.ascontiguousarray(np.asarray(qkvl_w, dtype=np.float32)),
        "qkvl_b": np.ascontiguousarray(np.asarray(qkvl_b, dtype=np.float32)),
    }
    nc = _get_nc()
    in_maps = [
        {"x": x[c * BPC:(c + 1) * BPC], **shared} for c in range(N_CORES)
    ]
    res = bass_utils.run_bass_kernel_spmd(nc, in_maps,
                                          core_ids=list(range(N_CORES)))
    return np.concatenate(
        [np.asarray(res.results[c]["out"]).astype(np.float32)
         for c in range(N_CORES)], axis=0)


# revision 36
# speedup vs baseline: 1.0555x; 1.0555x over previous
"""Trainium2 Bass kernel for the ELGCA block (dwconv3x3+gelu || conv1x1+gelu
-> pooled linear attention), data-parallel over batch on 8 NeuronCores.

Self-contained: hardcodes shapes B=16, C=128, H=W=128, f32 I/O.
kernel(**inputs) takes full unsharded inputs, returns the FULL f32 output.

v8 (per core, BPC=2 local images, partitions p = b*64 + c):
  - fp16 end-to-end on the matmul paths: the host pre-converts x to fp16
    (halves HBM read traffic, removes all on-chip casts); every PE matmul
    runs at 1 cycle/row.  A-side (q|k) precision in fp16 gives ~1.2e-2
    rel err end-to-end (bf16 fails at 0.34: softmax-logit amplification
    needs >=10 mantissa bits).  Pooling / qk / softmax stay f32.
  - dwconv3x3: 7 taps on PE as diagonal fp16 matmuls (tap-major LDW),
    2 taps (dx=1) on DVE in 4x/2x perf mode over the flat padded slab,
    merged via an identity matmul.  dwconv PSUM is half-window double
    buffered so next window's taps never wait on this window's gelu.
  - PE kept at HAM K=8/8: warm-up matmuls at start + through the serial
    softmax stretch; per-window emission order keeps the PE queue fed.
  - DMA: bulk stores (x1, out2) + slab loads on gpsimd SWDGE (spreads
    across all 16 SDMA engines; HWDGE rings degrade to ~2 engines for
    stores), x2 loads on scalar HWDGE, l stores on sync HWDGE.
  - outputs written fp16, widened to f32 on the host.
"""

import numpy as np
from contextlib import ExitStack

import concourse.bass as bass
import concourse.tile as tile
from concourse import bacc, mybir
from concourse import bass_utils
from concourse.masks import make_identity

F32 = mybir.dt.float32
FP16 = mybir.dt.float16
AX = mybir.AxisListType
ALU = mybir.AluOpType
ACTF = mybir.ActivationFunctionType

N_CORES = 8
B_TOT, C, H, W = 16, 128, 128, 128
BPC = B_TOT // N_CORES          # 2 images per core
HW = H * W                      # 16384
C2 = C // 2                     # 64
C4 = C // 4                     # 32
WP = W + 2                      # padded row width (130)
NW = 8                          # number of 16-row windows
WR = H // NW                    # image rows per window (16)
NP = (H // 2) * (W // 2)        # 4096 pooled positions
W2 = W // 2                     # 64
FLAT = 18 * WP                  # flat padded slab size (2340)

# taps: index t = dy*3+dx; PE takes 7, DVE takes 2 (dx=1 keeps the flat
# contiguous offset 4B-aligned for the DVE 2x/4x perf modes)
PE_TAPS = [(0, 0), (0, 1), (0, 2), (1, 0), (1, 2), (2, 0), (2, 2)]


def build_nc(loops=1):
    nc = bacc.Bacc("TRN2", target_bir_lowering=False, debug=False,
                   num_devices=N_CORES)
    x = nc.dram_tensor("x", [BPC, C, H, W], FP16, kind="ExternalInput").ap()
    dw_w = nc.dram_tensor("dw_w", [C2, 1, 3, 3], F32, kind="ExternalInput").ap()
    dw_b = nc.dram_tensor("dw_b", [C2], F32, kind="ExternalInput").ap()
    qw = nc.dram_tensor("qkvl_w", [C, C2, 1, 1], F32, kind="ExternalInput").ap()
    qb = nc.dram_tensor("qkvl_b", [C], F32, kind="ExternalInput").ap()
    out = nc.dram_tensor("out", [BPC, C, H, W], FP16, kind="ExternalOutput").ap()

    with tile.TileContext(nc) as tc, ExitStack() as ctx:
        consts = ctx.enter_context(tc.tile_pool(name="consts", bufs=1))
        inp = ctx.enter_context(tc.tile_pool(name="inp", bufs=4))
        bigp = ctx.enter_context(tc.tile_pool(name="bigp", bufs=1))
        stgp = ctx.enter_context(tc.tile_pool(name="stgp", bufs=2))
        ps = ctx.enter_context(tc.tile_pool(name="ps", bufs=1, space="PSUM"))

        id_f32 = consts.tile([128, 128], F32)
        make_identity(nc, id_f32[:])

        def issue_inputs(w):
            """DMA window w's inputs and return (slab, slab3, x2t)."""
            y0 = w * WR
            ys = max(y0 - 1, 0)
            ye = min(y0 + WR + 1, H)
            nrows = ye - ys
            rs = 0 if w > 0 else 1
            slab = inp.tile([128, FLAT], FP16, tag="slab")
            slab3 = slab.rearrange("p (r w) -> p r w", w=WP)
            nc.vector.memset(slab3[:, :, 0:1], 0.0)
            nc.vector.memset(slab3[:, :, WP - 1:WP], 0.0)
            if w == 0:
                nc.vector.memset(slab3[:, 0:1, :], 0.0)
            if w == NW - 1:
                nc.vector.memset(slab3[:, 17:18, :], 0.0)
            for b in range(BPC):
                nc.gpsimd.dma_start(
                    slab3[C2 * b:C2 * b + C2, rs:rs + nrows, 1:W + 1],
                    x[b:b + 1, 0:C2, ys:ye, :])
            x2t = inp.tile([128, WR * W], FP16, tag="x2t")
            x2t3 = x2t.rearrange("p (r w) -> p r w", w=W)
            nc.gpsimd.dma_start(x2t3[:, :, :],
                                x[0:BPC, C2:C, y0:y0 + WR, :])
            return slab, slab3, x2t

        # input DMAs for the first windows go out before the consts
        # chain occupies the queues
        pend = [issue_inputs(0), issue_inputs(1), issue_inputs(2)]

        # warm the PE (HAM throttle) while the first slab DMA is in
        # flight: dead accumulating matmuls on the identity
        wup = ps.tile([128, 1024], F32, tag="cv", bufs=2)
        for i in range(20):
            nc.tensor.matmul(wup[:, 0:128], id_f32[:], id_f32[:],
                             start=(i == 0), stop=(i == 19))

        # ---------------- constants (sync-queue loads) ----------------
        w_tile = consts.tile([128, 9], F32)
        dw9 = dw_w.rearrange("c o kh kw -> c (o kh kw)")
        nc.sync.dma_start(w_tile[0:C2, :], dw9)
        nc.sync.dma_start(w_tile[C2:128, :], dw9)

        # qkvl_w: load [128oc, 64ic] contiguous, PE-transpose to [64ic, 128oc]
        qw_oc = consts.tile([128, C2], F32)
        nc.sync.dma_start(qw_oc[:], qw.rearrange("o i kh kw -> o (i kh kw)"))
        qwT_ps = ps.tile([128, 1024], F32, tag="cv", bufs=2)
        nc.tensor.transpose(qwT_ps[0:C2, 0:128], qw_oc[:], id_f32[:])
        qwT_sb = consts.tile([C2, 128], F32)
        nc.scalar.copy(qwT_sb[:], qwT_ps[0:C2, 0:128])

        # row-vector loads (1 descriptor each) for biases
        qb_row = consts.tile([1, C], F32)
        nc.sync.dma_start(qb_row[:], qb.unsqueeze(0))
        dwb_row = consts.tile([1, C2], F32)
        nc.sync.dma_start(dwb_row[:], dw_b.unsqueeze(0))

        # permuted bias rows -> PE transpose to per-partition columns
        # brow col-blocks: biasA = [qb0:32|qb0:32|qb32:64|qb32:64],
        # biasB = [qb64:96|...], dwb = [dwb|dwb]
        brow = consts.tile([1, 3 * 128], F32)
        nc.scalar.copy(brow[:, 0:C4], qb_row[:, 0:C4])
        nc.scalar.copy(brow[:, C4:C2], qb_row[:, 0:C4])
        nc.scalar.copy(brow[:, C2:96], qb_row[:, C4:C2])
        nc.scalar.copy(brow[:, 96:128], qb_row[:, C4:C2])
        nc.scalar.copy(brow[:, 128:160], qb_row[:, C2:96])
        nc.scalar.copy(brow[:, 160:192], qb_row[:, C2:96])
        nc.scalar.copy(brow[:, 192:224], qb_row[:, 96:128])
        nc.scalar.copy(brow[:, 224:256], qb_row[:, 96:128])
        nc.scalar.copy(brow[:, 256:320], dwb_row[:, 0:C2])
        nc.scalar.copy(brow[:, 320:384], dwb_row[:, 0:C2])
        bcol_ps = ps.tile([128, 1024], F32, tag="dw", bufs=2)
        for i in range(3):
            nc.tensor.transpose(bcol_ps[:, i:i + 1],
                                brow[0:1, i * 128:(i + 1) * 128],
                                id_f32[0:1, 0:1])
        bcol = consts.tile([128, 3], F32)
        nc.scalar.copy(bcol[:], bcol_ps[:, 0:3])
        biasA = bcol[:, 0:1]
        biasB = bcol[:, 1:2]
        dwb_t = bcol[:, 2:3]

        # 9 diagonal tap matrices, fp16 (direct f32->fp16 on DVE)
        wdiag = consts.tile([128, 9 * 128], FP16)
        for t in range(9):
            nc.vector.tensor_scalar_mul(
                wdiag[:, t * 128:(t + 1) * 128], id_f32[:],
                w_tile[:, t:t + 1])

        # conv1x1 block-diagonal weights from qwT_sb (direct fp16 copies)
        lhsAB = consts.tile([128, 256], FP16)
        nc.vector.memset(lhsAB[:], 0.0)
        nc.scalar.copy(lhsAB[0:C2, 0:C4], qwT_sb[:, 0:C4])
        nc.scalar.copy(lhsAB[C2:128, C4:C2], qwT_sb[:, 0:C4])
        nc.scalar.copy(lhsAB[0:C2, C2:96], qwT_sb[:, C4:C2])
        nc.scalar.copy(lhsAB[C2:128, 96:128], qwT_sb[:, C4:C2])
        nc.scalar.copy(lhsAB[0:C2, 128:160], qwT_sb[:, C2:96])
        nc.scalar.copy(lhsAB[C2:128, 160:192], qwT_sb[:, C2:96])
        nc.scalar.copy(lhsAB[0:C2, 192:224], qwT_sb[:, 96:128])
        nc.scalar.copy(lhsAB[C2:128, 224:256], qwT_sb[:, 96:128])
        id_h = consts.tile([128, 128], FP16)
        nc.vector.tensor_copy(id_h[:], id_f32[:])
        lhsA = lhsAB[:, 0:128]
        lhsB = lhsAB[:, 128:256]

        def one_pass():
            # persistent per-pass buffers
            vl = bigp.tile([128, HW], FP16, tag="vl")   # v rows 0:64, l 64:128
            hp = bigp.tile([128, H * W2], F32, tag="hp")
            hp3 = hp.rearrange("p (r w) -> p r w", w=W2)
            hpv = hp.rearrange("p (o two w) -> p o two w", two=2, w=W2)
            pp = bigp.tile([128, NP], F32, tag="pp")
            pp3 = pp.rearrange("p (r w) -> p r w", w=W2)
            qk_acc = bigp.tile([C2, C2], F32, tag="qk")

            def attn_pp(sw):
                """v-pool pooled rows [8sw, 8sw+8) (DVE part of attn)."""
                o0 = 8 * sw
                nc.vector.tensor_add(pp3[0:C2, o0:o0 + 8, :],
                                     hpv[0:C2, o0:o0 + 8, 0, :],
                                     hpv[0:C2, o0:o0 + 8, 1, :])
                lo = max(o0, 1)
                nc.vector.tensor_add(pp3[0:C2, lo:o0 + 8, :],
                                     pp3[0:C2, lo:o0 + 8, :],
                                     hpv[0:C2, lo - 1:o0 + 7, 1, :])
                nc.vector.tensor_max(pp3[C2:128, o0:o0 + 8, :],
                                     hpv[C2:128, o0:o0 + 8, 0, :],
                                     hpv[C2:128, o0:o0 + 8, 1, :])

            def attn_tr(sw):
                """transpose the 4 new 128-position chunks."""
                trps = ps.tile([128, 1024], F32, tag="cv", bufs=2)
                for jj in range(4):
                    ch = 4 * sw + jj
                    nc.tensor.transpose(trps[:, jj * 128:(jj + 1) * 128],
                                        pp[:, ch * 128:(ch + 1) * 128],
                                        id_f32[:])
                trsb = stgp.tile([128, 512], FP16, tag="trsb")
                nc.scalar.copy(trsb[:], trps[:, 0:512])
                return trsb

            def attn_qk(sw, trsb):
                """accumulate this slice's qk partial."""
                qkps = ps.tile([128, 1024], F32, tag="cv", bufs=2)
                for jj in range(4):
                    nc.tensor.matmul(
                        qkps[0:C2, 0:C2],
                        trsb[:, jj * 128 + C2:(jj + 1) * 128],
                        trsb[:, jj * 128:jj * 128 + C2],
                        start=(jj == 0), stop=(jj == 3))
                if sw == 0:
                    nc.scalar.copy(qk_acc[:], qkps[0:C2, 0:C2])
                else:
                    qktmp = stgp.tile([C2, C2], F32, tag="qktmp")
                    nc.vector.tensor_copy(qktmp[:], qkps[0:C2, 0:C2])
                    nc.gpsimd.tensor_add(qk_acc[:], qk_acc[:], qktmp[:])

            for w in range(NW):
                y0 = w * WR
                slab, slab3, x2t = pend.pop(0)
                if w + 3 < NW:
                    pend.append(issue_inputs(w + 3))

                qg = stgp.tile([128, WR * W], F32, tag="qg", bufs=1)
                x1st = stgp.tile([128, WR * W], FP16, tag="x1st")

                # ---- PE dwconv taps first: keeps the PE streaming while
                # DVE computes the dx=1 taps and the attn pooling.
                # Tap-major across both halves: one LDWEIGHTS per tap. ----
                dwps = []
                for _h in range(2):
                    dwtile = ps.tile([128, 1024], F32, tag="dw", bufs=2)
                    dwps.append(dwtile)
                for ti, (dy, dx) in enumerate(PE_TAPS):
                    t = dy * 3 + dx
                    for h in range(2):
                        for qq in range(2):
                            la = h * 8 + qq * 4
                            nc.tensor.matmul(
                                dwps[h][:, qq * 512:(qq + 1) * 512],
                                wdiag[:, t * 128:(t + 1) * 128],
                                slab3[:, la + dy:la + dy + 4, dx:dx + W],
                                start=(ti == 0), stop=False)

                # ---- attn pooling + transposes for the previous window ----
                trsb = None
                if w >= 1:
                    attn_pp(w - 1)
                    trsb = attn_tr(w - 1)

                # ---- dx=1 dwconv taps on DVE (flat contiguous, 4x/2x) ----
                dwacc = stgp.tile([128, 16 * WP], FP16, tag="dwacc", bufs=1)
                dwtmp = stgp.tile([128, 16 * WP], FP16, tag="dwtmp", bufs=1)
                nc.vector.tensor_scalar_mul(
                    dwacc[:, 0:16 * WP], slab[:, WP:WP + 16 * WP],
                    w_tile[:, 4:5])
                nc.vector.tensor_scalar_mul(
                    dwtmp[:, 0:16 * WP], slab[:, 2 * WP:2 * WP + 16 * WP],
                    w_tile[:, 7:8])
                nc.vector.tensor_add(dwacc[:, 0:16 * WP],
                                     dwacc[:, 0:16 * WP],
                                     dwtmp[:, 0:16 * WP])
                dwacc3 = dwacc.rearrange("p (r w) -> p r w", w=WP)

                # ---- conv1x1 A pairs (fp16 -> f32 qg) ----
                for pr in range(2):
                    pc0 = pr * 1024
                    Aps = ps.tile([128, 1024], F32, tag="cv", bufs=2)
                    for hf in range(2):
                        nc.tensor.matmul(
                            Aps[:, hf * 512:(hf + 1) * 512], lhsA,
                            x2t[:, pc0 + hf * 512:pc0 + (hf + 1) * 512],
                            start=True, stop=True)
                    nc.scalar.activation(qg[:, pc0:pc0 + 1024], Aps[:],
                                         ACTF.Gelu, bias=biasA)

                # ---- attn qk for the previous window ----
                if w >= 1:
                    attn_qk(w - 1, trsb)

                # ---- conv1x1 B pairs -> vl ----
                for pr in range(2):
                    pc0 = pr * 1024
                    Bps = ps.tile([128, 1024], F32, tag="cv", bufs=2)
                    for hf in range(2):
                        nc.tensor.matmul(
                            Bps[:, hf * 512:(hf + 1) * 512], lhsB,
                            x2t[:, pc0 + hf * 512:pc0 + (hf + 1) * 512],
                            start=True, stop=True)
                    nc.scalar.activation(vl[:, y0 * W + pc0:
                                            y0 * W + pc0 + 1024],
                                         Bps[:], ACTF.Gelu, bias=biasB)

                # ---- merge DVE taps into the PE accumulators, gelu ----
                for h in range(2):
                    dw = dwps[h]
                    for qq in range(2):
                        la = h * 8 + qq * 4
                        nc.tensor.matmul(
                            dw[:, qq * 512:(qq + 1) * 512], id_h[:],
                            dwacc3[:, la:la + 4, 1:W + 1],
                            start=False, stop=True)
                    nc.scalar.activation(x1st[:, h * 1024:(h + 1) * 1024],
                                         dw[:], ACTF.Gelu, bias=dwb_t)
                x1st3 = x1st.rearrange("p (r w) -> p r w", w=W)
                for b in range(BPC):
                    nc.scalar.dma_start(out[b:b + 1, 0:C2, y0:y0 + WR, :],
                                        x1st3[C2 * b:C2 * b + C2, :, :])

                # ---- l output DMA for this window ----
                nc.gpsimd.dma_start(
                    out[0:BPC, C2:96, y0:y0 + WR, :],
                    vl[C2:128, y0 * W:(y0 + WR) * W]
                    .rearrange("p (r w) -> p r w", w=W))

                # ---- horizontal pooling for this window ----
                qg3 = qg.rearrange("p (r w2 two) -> p r w2 two", two=2, w2=W2)
                nc.vector.tensor_add(hp3[0:C2, y0:y0 + WR, :],
                                     qg3[0:C2, :, :, 0], qg3[0:C2, :, :, 1])
                nc.vector.tensor_add(hp3[0:C2, y0:y0 + WR, 1:W2],
                                     hp3[0:C2, y0:y0 + WR, 1:W2],
                                     qg3[0:C2, :, 0:W2 - 1, 1])
                nc.vector.tensor_max(hp3[C2:128, y0:y0 + WR, :],
                                     qg3[C2:128, :, :, 0],
                                     qg3[C2:128, :, :, 1])

            attn_pp(NW - 1)
            attn_qk(NW - 1, attn_tr(NW - 1))

            # keep the PE (HAM) warm through the serial softmax stretch:
            # dead accumulating matmuls, emitted BEFORE the Ebd transpose
            # so they sit ahead of it in the PE's in-order queue
            warmps = ps.tile([128, 1024], F32, tag="cv", bufs=2)
            for i in range(24):
                nc.tensor.matmul(warmps[:, 0:512], id_h[:],
                                 vl[:, i * 512:(i + 1) * 512],
                                 start=(i == 0), stop=(i == 23))

            # ---------- softmax stats -> block-diag attention ----------
            # Both image blocks in one pass: mask off-diagonal blocks to
            # -inf so the row reduce/exp/sum ignore them (exp -> 0, which
            # also zeroes the off-blocks of Ebd for free).
            Ebd = bigp.tile([C2, C2], FP16, tag="Ebd")
            qk9 = bigp.tile([C2, C2], F32, tag="qk9")
            nc.scalar.mul(qk9[:], qk_acc[:], 1.0 / 9.0)
            nc.vector.memset(qk9[0:C4, C4:C2], -1e30)
            nc.vector.memset(qk9[C4:C2, 0:C4], -1e30)
            nmax = bigp.tile([C2, 1], F32, tag="nmax")
            nc.vector.tensor_reduce(nmax[:], qk9[:], axis=AX.X,
                                    op=ALU.max, negate=True)
            ET = bigp.tile([C2, C2], F32, tag="ET")
            nc.scalar.activation(ET[:], qk9[:], ACTF.Exp,
                                 bias=nmax[:, 0:1])
            ssum = bigp.tile([C2, 1], F32, tag="ssum")
            nc.vector.reduce_sum(ssum[:], ET[:], axis=AX.X)
            rec = bigp.tile([C2, 1], F32, tag="rec")
            nc.vector.reciprocal(rec[:], ssum[:])
            ETn = bigp.tile([C2, C2], F32, tag="ETn")
            nc.vector.tensor_scalar_mul(ETn[:], ET[:], rec[:, 0:1])
            etp = ps.tile([128, 1024], F32, tag="dw", bufs=2)
            nc.tensor.transpose(etp[0:C2, 0:C2], ETn[:],
                                id_f32[0:C2, 0:C2])
            nc.scalar.copy(Ebd[:], etp[0:C2, 0:C2])

            # ---------- out2 = attn @ v, both batches per matmul ----------
            for w in range(NW):
                y0 = w * WR
                o2st = stgp.tile([C2, WR * W], FP16, tag="o2st", bufs=8)
                for pr in range(2):
                    pc0 = pr * 1024
                    o2ps = ps.tile([128, 1024], F32,
                                   tag=("cv" if pr == 0 else "dw"), bufs=2)
                    for hf in range(2):
                        nc.tensor.matmul(
                            o2ps[0:C2, hf * 512:(hf + 1) * 512], Ebd[:],
                            vl[0:C2, y0 * W + pc0 + hf * 512:
                               y0 * W + pc0 + (hf + 1) * 512],
                            start=True, stop=True)
                    if pr == 0:
                        nc.scalar.copy(o2st[:, pc0:pc0 + 1024],
                                       o2ps[0:C2, :])
                    else:
                        nc.vector.tensor_copy(o2st[:, pc0:pc0 + 1024],
                                              o2ps[0:C2, :])
                o2st3 = o2st.rearrange("p (r w) -> p r w", w=W)
                nc.gpsimd.dma_start(
                    out[0:BPC, 96:128, y0:y0 + WR, :],
                    o2st3[:, :, :])

        for _ in range(loops):
            one_pass()

    nc.compile()
    return nc


_NC_CACHE = None


def _get_nc():
    global _NC_CACHE
    if _NC_CACHE is None:
        _NC_CACHE = build_nc()
    return _NC_CACHE


def kernel(x, dw_w, dw_b, qkvl_w, qkvl_b):
    x = np.ascontiguousarray(np.asarray(x).astype(np.float16))
    shared = {
        "dw_w": np.ascontiguousarray(np.asarray(dw_w, dtype=np.float32)),
        "dw_b": np.ascontiguousarray(np.asarray(dw_b, dtype=np.float32)),
        "qkvl_w": np.ascontiguousarray(np.asarray(qkvl_w, dtype=np.float32)),
        "qkvl_b": np.ascontiguousarray(np.asarray(qkvl_b, dtype=np.float32)),
    }
    nc = _get_nc()
    in_maps = [
        {"x": x[c * BPC:(c + 1) * BPC], **shared} for c in range(N_CORES)
    ]
    res = bass_utils.run_bass_kernel_spmd(nc, in_maps,
                                          core_ids=list(range(N_CORES)))
    return np.concatenate(
        [np.asarray(res.results[c]["out"]).astype(np.float32)
         for c in range(N_CORES)], axis=0)


# revision 37
# speedup vs baseline: 1.1212x; 1.0622x over previous
"""Trainium2 Bass kernel for the ELGCA block (dwconv3x3+gelu || conv1x1+gelu
-> pooled linear attention), data-parallel over batch on 8 NeuronCores.

Self-contained: hardcodes shapes B=16, C=128, H=W=128, f32 I/O.
kernel(**inputs) takes full unsharded inputs, returns the FULL f32 output.

v8 (per core, BPC=2 local images, partitions p = b*64 + c):
  - fp16 end-to-end on the matmul paths: the host pre-converts x to fp16
    (halves HBM read traffic, removes all on-chip casts); every PE matmul
    runs at 1 cycle/row.  A-side (q|k) precision in fp16 gives ~1.2e-2
    rel err end-to-end (bf16 fails at 0.34: softmax-logit amplification
    needs >=10 mantissa bits).  Pooling / qk / softmax stay f32.
  - dwconv3x3: 7 taps on PE as diagonal fp16 matmuls (tap-major LDW),
    2 taps (dx=1) on DVE in 4x/2x perf mode over the flat padded slab,
    merged via an identity matmul.  dwconv PSUM is half-window double
    buffered so next window's taps never wait on this window's gelu.
  - PE kept at HAM K=8/8: warm-up matmuls at start + through the serial
    softmax stretch; per-window emission order keeps the PE queue fed.
  - DMA: bulk stores (x1, out2) + slab loads on gpsimd SWDGE (spreads
    across all 16 SDMA engines; HWDGE rings degrade to ~2 engines for
    stores), x2 loads on scalar HWDGE, l stores on sync HWDGE.
  - outputs written fp16, widened to f32 on the host.
"""

import numpy as np
from contextlib import ExitStack

import concourse.bass as bass
import concourse.tile as tile
from concourse import bacc, mybir
from concourse import bass_utils
from concourse.masks import make_identity

F32 = mybir.dt.float32
FP16 = mybir.dt.float16
AX = mybir.AxisListType
ALU = mybir.AluOpType
ACTF = mybir.ActivationFunctionType

N_CORES = 8
B_TOT, C, H, W = 16, 128, 128, 128
BPC = B_TOT // N_CORES          # 2 images per core
HW = H * W                      # 16384
C2 = C // 2                     # 64
C4 = C // 4                     # 32
WP = W + 2                      # padded row width (130)
NW = 8                          # number of 16-row windows
WR = H // NW                    # image rows per window (16)
NP = (H // 2) * (W // 2)        # 4096 pooled positions
W2 = W // 2                     # 64
FLAT = 18 * WP                  # flat padded slab size (2340)

# taps: index t = dy*3+dx; PE takes 7, DVE takes 2 (dx=1 keeps the flat
# contiguous offset 4B-aligned for the DVE 2x/4x perf modes)
PE_TAPS = [(0, 0), (0, 1), (0, 2), (1, 0), (1, 2), (2, 0), (2, 2)]


def build_nc(loops=1):
    nc = bacc.Bacc("TRN2", target_bir_lowering=False, debug=False,
                   num_devices=N_CORES)
    x = nc.dram_tensor("x", [BPC, C, H, W], FP16, kind="ExternalInput").ap()
    dw_w = nc.dram_tensor("dw_w", [C2, 1, 3, 3], F32, kind="ExternalInput").ap()
    dw_b = nc.dram_tensor("dw_b", [C2], F32, kind="ExternalInput").ap()
    qw = nc.dram_tensor("qkvl_w", [C, C2, 1, 1], F32, kind="ExternalInput").ap()
    qb = nc.dram_tensor("qkvl_b", [C], F32, kind="ExternalInput").ap()
    out = nc.dram_tensor("out", [BPC, C, H, W], FP16, kind="ExternalOutput").ap()

    with tile.TileContext(nc) as tc, ExitStack() as ctx:
        consts = ctx.enter_context(tc.tile_pool(name="consts", bufs=1))
        inp = ctx.enter_context(tc.tile_pool(name="inp", bufs=4))
        bigp = ctx.enter_context(tc.tile_pool(name="bigp", bufs=1))
        stgp = ctx.enter_context(tc.tile_pool(name="stgp", bufs=2))
        ps = ctx.enter_context(tc.tile_pool(name="ps", bufs=1, space="PSUM"))

        id_f32 = consts.tile([128, 128], F32)
        make_identity(nc, id_f32[:])

        def issue_inputs(w):
            """DMA window w's inputs and return (slab, slab3, x2t)."""
            y0 = w * WR
            ys = max(y0 - 1, 0)
            ye = min(y0 + WR + 1, H)
            nrows = ye - ys
            rs = 0 if w > 0 else 1
            slab = inp.tile([128, FLAT], FP16, tag="slab")
            slab3 = slab.rearrange("p (r w) -> p r w", w=WP)
            nc.vector.memset(slab3[:, :, 0:1], 0.0)
            nc.vector.memset(slab3[:, :, WP - 1:WP], 0.0)
            if w == 0:
                nc.vector.memset(slab3[:, 0:1, :], 0.0)
            if w == NW - 1:
                nc.vector.memset(slab3[:, 17:18, :], 0.0)
            for b in range(BPC):
                nc.gpsimd.dma_start(
                    slab3[C2 * b:C2 * b + C2, rs:rs + nrows, 1:W + 1],
                    x[b:b + 1, 0:C2, ys:ye, :])
            x2t = inp.tile([128, WR * W], FP16, tag="x2t")
            x2t3 = x2t.rearrange("p (r w) -> p r w", w=W)
            nc.gpsimd.dma_start(x2t3[:, :, :],
                                x[0:BPC, C2:C, y0:y0 + WR, :])
            return slab, slab3, x2t

        # input DMAs for the first windows go out before the consts
        # chain occupies the queues
        pend = [issue_inputs(0), issue_inputs(1), issue_inputs(2)]

        # warm the PE (HAM throttle) while the first slab DMA is in
        # flight: dead accumulating matmuls on the identity
        wup = ps.tile([128, 1024], F32, tag="cv", bufs=2)
        for i in range(20):
            nc.tensor.matmul(wup[:, 0:128], id_f32[:], id_f32[:],
                             start=(i == 0), stop=(i == 19))

        # ---------------- constants (sync-queue loads) ----------------
        w_tile = consts.tile([128, 9], F32)
        dw9 = dw_w.rearrange("c o kh kw -> c (o kh kw)")
        nc.sync.dma_start(w_tile[0:C2, :], dw9)
        nc.sync.dma_start(w_tile[C2:128, :], dw9)

        # qkvl_w: load [128oc, 64ic] contiguous, PE-transpose to [64ic, 128oc]
        qw_oc = consts.tile([128, C2], F32)
        nc.sync.dma_start(qw_oc[:], qw.rearrange("o i kh kw -> o (i kh kw)"))
        qwT_ps = ps.tile([128, 1024], F32, tag="cv", bufs=2)
        nc.tensor.transpose(qwT_ps[0:C2, 0:128], qw_oc[:], id_f32[:])
        qwT_sb = consts.tile([C2, 128], F32)
        nc.scalar.copy(qwT_sb[:], qwT_ps[0:C2, 0:128])

        # row-vector loads (1 descriptor each) for biases
        qb_row = consts.tile([1, C], F32)
        nc.sync.dma_start(qb_row[:], qb.unsqueeze(0))
        dwb_row = consts.tile([1, C2], F32)
        nc.sync.dma_start(dwb_row[:], dw_b.unsqueeze(0))

        # permuted bias rows -> PE transpose to per-partition columns
        # brow col-blocks: biasA = [qb0:32|qb0:32|qb32:64|qb32:64],
        # biasB = [qb64:96|...], dwb = [dwb|dwb]
        brow = consts.tile([1, 3 * 128], F32)
        nc.scalar.copy(brow[:, 0:C4], qb_row[:, 0:C4])
        nc.scalar.copy(brow[:, C4:C2], qb_row[:, 0:C4])
        nc.scalar.copy(brow[:, C2:96], qb_row[:, C4:C2])
        nc.scalar.copy(brow[:, 96:128], qb_row[:, C4:C2])
        nc.scalar.copy(brow[:, 128:160], qb_row[:, C2:96])
        nc.scalar.copy(brow[:, 160:192], qb_row[:, C2:96])
        nc.scalar.copy(brow[:, 192:224], qb_row[:, 96:128])
        nc.scalar.copy(brow[:, 224:256], qb_row[:, 96:128])
        nc.scalar.copy(brow[:, 256:320], dwb_row[:, 0:C2])
        nc.scalar.copy(brow[:, 320:384], dwb_row[:, 0:C2])
        bcol_ps = ps.tile([128, 1024], F32, tag="dw", bufs=2)
        for i in range(3):
            nc.tensor.transpose(bcol_ps[:, i:i + 1],
                                brow[0:1, i * 128:(i + 1) * 128],
                                id_f32[0:1, 0:1])
        bcol = consts.tile([128, 3], F32)
        nc.scalar.copy(bcol[:], bcol_ps[:, 0:3])
        biasA = bcol[:, 0:1]
        biasB = bcol[:, 1:2]
        dwb_t = bcol[:, 2:3]

        # 9 diagonal tap matrices, fp16 (direct f32->fp16 on DVE)
        wdiag = consts.tile([128, 9 * 128], FP16)
        for t in range(9):
            nc.vector.tensor_scalar_mul(
                wdiag[:, t * 128:(t + 1) * 128], id_f32[:],
                w_tile[:, t:t + 1])

        # conv1x1 block-diagonal weights from qwT_sb (direct fp16 copies)
        lhsAB = consts.tile([128, 256], FP16)
        nc.vector.memset(lhsAB[:], 0.0)
        nc.scalar.copy(lhsAB[0:C2, 0:C4], qwT_sb[:, 0:C4])
        nc.scalar.copy(lhsAB[C2:128, C4:C2], qwT_sb[:, 0:C4])
        nc.scalar.copy(lhsAB[0:C2, C2:96], qwT_sb[:, C4:C2])
        nc.scalar.copy(lhsAB[C2:128, 96:128], qwT_sb[:, C4:C2])
        nc.scalar.copy(lhsAB[0:C2, 128:160], qwT_sb[:, C2:96])
        nc.scalar.copy(lhsAB[C2:128, 160:192], qwT_sb[:, C2:96])
        nc.scalar.copy(lhsAB[0:C2, 192:224], qwT_sb[:, 96:128])
        nc.scalar.copy(lhsAB[C2:128, 224:256], qwT_sb[:, 96:128])
        id_h = consts.tile([128, 128], FP16)
        nc.vector.tensor_copy(id_h[:], id_f32[:])
        lhsA = lhsAB[:, 0:128]
        lhsB = lhsAB[:, 128:256]

        def one_pass():
            # persistent per-pass buffers
            vl = bigp.tile([128, HW], FP16, tag="vl")   # v rows 0:64, l 64:128
            hp = bigp.tile([128, H * W2], F32, tag="hp")
            hp3 = hp.rearrange("p (r w) -> p r w", w=W2)
            hpv = hp.rearrange("p (o two w) -> p o two w", two=2, w=W2)
            pp = bigp.tile([128, NP], F32, tag="pp")
            pp3 = pp.rearrange("p (r w) -> p r w", w=W2)
            qk_acc = bigp.tile([C2, C2], F32, tag="qk")

            def attn_pp(sw):
                """v-pool pooled rows [8sw, 8sw+8) (DVE part of attn)."""
                o0 = 8 * sw
                nc.vector.tensor_add(pp3[0:C2, o0:o0 + 8, :],
                                     hpv[0:C2, o0:o0 + 8, 0, :],
                                     hpv[0:C2, o0:o0 + 8, 1, :])
                lo = max(o0, 1)
                nc.vector.tensor_add(pp3[0:C2, lo:o0 + 8, :],
                                     pp3[0:C2, lo:o0 + 8, :],
                                     hpv[0:C2, lo - 1:o0 + 7, 1, :])
                nc.vector.tensor_max(pp3[C2:128, o0:o0 + 8, :],
                                     hpv[C2:128, o0:o0 + 8, 0, :],
                                     hpv[C2:128, o0:o0 + 8, 1, :])

            def attn_tr(sw):
                """transpose the 4 new 128-position chunks."""
                trps = ps.tile([128, 1024], F32, tag="cv", bufs=2)
                for jj in range(4):
                    ch = 4 * sw + jj
                    nc.tensor.transpose(trps[:, jj * 128:(jj + 1) * 128],
                                        pp[:, ch * 128:(ch + 1) * 128],
                                        id_f32[:])
                trsb = stgp.tile([128, 512], FP16, tag="trsb")
                nc.scalar.copy(trsb[:], trps[:, 0:512])
                return trsb

            def attn_qk(sw, trsb):
                """accumulate this slice's qk partial."""
                qkps = ps.tile([128, 1024], F32, tag="cv", bufs=2)
                for jj in range(4):
                    nc.tensor.matmul(
                        qkps[0:C2, 0:C2],
                        trsb[:, jj * 128 + C2:(jj + 1) * 128],
                        trsb[:, jj * 128:jj * 128 + C2],
                        start=(jj == 0), stop=(jj == 3))
                if sw == 0:
                    nc.scalar.copy(qk_acc[:], qkps[0:C2, 0:C2])
                else:
                    qktmp = stgp.tile([C2, C2], F32, tag="qktmp")
                    nc.scalar.copy(qktmp[:], qkps[0:C2, 0:C2])
                    nc.gpsimd.tensor_add(qk_acc[:], qk_acc[:], qktmp[:])

            for w in range(NW):
                y0 = w * WR
                slab, slab3, x2t = pend.pop(0)
                if w + 3 < NW:
                    pend.append(issue_inputs(w + 3))

                qg = stgp.tile([128, WR * W], F32, tag="qg", bufs=1)
                x1st = stgp.tile([128, WR * W], FP16, tag="x1st")

                # ---- PE dwconv taps first: keeps the PE streaming while
                # DVE computes the dx=1 taps and the attn pooling.
                # Tap-major across both halves: one LDWEIGHTS per tap. ----
                dwps = []
                for _h in range(2):
                    dwtile = ps.tile([128, 1024], F32, tag="dw", bufs=2)
                    dwps.append(dwtile)
                for ti, (dy, dx) in enumerate(PE_TAPS):
                    t = dy * 3 + dx
                    for h in range(2):
                        for qq in range(2):
                            la = h * 8 + qq * 4
                            nc.tensor.matmul(
                                dwps[h][:, qq * 512:(qq + 1) * 512],
                                wdiag[:, t * 128:(t + 1) * 128],
                                slab3[:, la + dy:la + dy + 4, dx:dx + W],
                                start=(ti == 0), stop=False)

                # ---- attn pooling + transposes for the previous window ----
                trsb = None
                if w >= 1:
                    attn_pp(w - 1)
                    trsb = attn_tr(w - 1)

                # ---- dx=1 dwconv taps on DVE (flat contiguous, 4x/2x) ----
                dwacc = stgp.tile([128, 16 * WP], FP16, tag="dwacc", bufs=1)
                dwtmp = stgp.tile([128, 16 * WP], FP16, tag="dwtmp", bufs=1)
                nc.vector.tensor_scalar_mul(
                    dwacc[:, 0:16 * WP], slab[:, WP:WP + 16 * WP],
                    w_tile[:, 4:5])
                nc.vector.tensor_scalar_mul(
                    dwtmp[:, 0:16 * WP], slab[:, 2 * WP:2 * WP + 16 * WP],
                    w_tile[:, 7:8])
                nc.vector.tensor_add(dwacc[:, 0:16 * WP],
                                     dwacc[:, 0:16 * WP],
                                     dwtmp[:, 0:16 * WP])
                dwacc3 = dwacc.rearrange("p (r w) -> p r w", w=WP)

                # ---- conv1x1 A pairs (fp16 -> f32 qg) ----
                for pr in range(2):
                    pc0 = pr * 1024
                    Aps = ps.tile([128, 1024], F32, tag="cv", bufs=2)
                    for hf in range(2):
                        nc.tensor.matmul(
                            Aps[:, hf * 512:(hf + 1) * 512], lhsA,
                            x2t[:, pc0 + hf * 512:pc0 + (hf + 1) * 512],
                            start=True, stop=True)
                    nc.scalar.activation(qg[:, pc0:pc0 + 1024], Aps[:],
                                         ACTF.Gelu, bias=biasA)

                # ---- attn qk for the previous window ----
                if w >= 1:
                    attn_qk(w - 1, trsb)

                # ---- conv1x1 B pairs -> vl ----
                for pr in range(2):
                    pc0 = pr * 1024
                    Bps = ps.tile([128, 1024], F32, tag="cv", bufs=2)
                    for hf in range(2):
                        nc.tensor.matmul(
                            Bps[:, hf * 512:(hf + 1) * 512], lhsB,
                            x2t[:, pc0 + hf * 512:pc0 + (hf + 1) * 512],
                            start=True, stop=True)
                    nc.scalar.activation(vl[:, y0 * W + pc0:
                                            y0 * W + pc0 + 1024],
                                         Bps[:], ACTF.Gelu, bias=biasB)

                # ---- merge DVE taps into the PE accumulators, gelu ----
                for h in range(2):
                    dw = dwps[h]
                    for qq in range(2):
                        la = h * 8 + qq * 4
                        nc.tensor.matmul(
                            dw[:, qq * 512:(qq + 1) * 512], id_h[:],
                            dwacc3[:, la:la + 4, 1:W + 1],
                            start=False, stop=True)
                    nc.scalar.activation(x1st[:, h * 1024:(h + 1) * 1024],
                                         dw[:], ACTF.Gelu, bias=dwb_t)
                x1st3 = x1st.rearrange("p (r w) -> p r w", w=W)
                for b in range(BPC):
                    nc.scalar.dma_start(out[b:b + 1, 0:C2, y0:y0 + WR, :],
                                        x1st3[C2 * b:C2 * b + C2, :, :])

                # ---- l output DMA for this window ----
                nc.gpsimd.dma_start(
                    out[0:BPC, C2:96, y0:y0 + WR, :],
                    vl[C2:128, y0 * W:(y0 + WR) * W]
                    .rearrange("p (r w) -> p r w", w=W))

                # ---- horizontal pooling for this window ----
                qg3 = qg.rearrange("p (r w2 two) -> p r w2 two", two=2, w2=W2)
                nc.vector.tensor_add(hp3[0:C2, y0:y0 + WR, :],
                                     qg3[0:C2, :, :, 0], qg3[0:C2, :, :, 1])
                nc.vector.tensor_add(hp3[0:C2, y0:y0 + WR, 1:W2],
                                     hp3[0:C2, y0:y0 + WR, 1:W2],
                                     qg3[0:C2, :, 0:W2 - 1, 1])
                nc.vector.tensor_max(hp3[C2:128, y0:y0 + WR, :],
                                     qg3[C2:128, :, :, 0],
                                     qg3[C2:128, :, :, 1])

            attn_pp(NW - 1)
            attn_qk(NW - 1, attn_tr(NW - 1))

            # keep the PE (HAM) warm through the serial softmax stretch:
            # dead accumulating matmuls, emitted BEFORE the Ebd transpose
            # so they sit ahead of it in the PE's in-order queue
            warmps = ps.tile([128, 1024], F32, tag="cv", bufs=2)
            for i in range(24):
                nc.tensor.matmul(warmps[:, 0:512], id_h[:],
                                 vl[:, i * 512:(i + 1) * 512],
                                 start=(i == 0), stop=(i == 23))

            # ---------- softmax stats -> block-diag attention ----------
            # Both image blocks in one pass: mask off-diagonal blocks to
            # -inf so the row reduce/exp/sum ignore them (exp -> 0, which
            # also zeroes the off-blocks of Ebd for free).
            Ebd = bigp.tile([C2, C2], FP16, tag="Ebd")
            qk9 = bigp.tile([C2, C2], F32, tag="qk9")
            nc.scalar.mul(qk9[:], qk_acc[:], 1.0 / 9.0)
            nc.vector.memset(qk9[0:C4, C4:C2], -1e30)
            nc.vector.memset(qk9[C4:C2, 0:C4], -1e30)
            nmax = bigp.tile([C2, 1], F32, tag="nmax")
            nc.vector.tensor_reduce(nmax[:], qk9[:], axis=AX.X,
                                    op=ALU.max, negate=True)
            ET = bigp.tile([C2, C2], F32, tag="ET")
            nc.scalar.activation(ET[:], qk9[:], ACTF.Exp,
                                 bias=nmax[:, 0:1])
            ssum = bigp.tile([C2, 1], F32, tag="ssum")
            nc.vector.reduce_sum(ssum[:], ET[:], axis=AX.X)
            rec = bigp.tile([C2, 1], F32, tag="rec")
            nc.vector.reciprocal(rec[:], ssum[:])
            ETn = bigp.tile([C2, C2], F32, tag="ETn")
            nc.vector.tensor_scalar_mul(ETn[:], ET[:], rec[:, 0:1])
            etp = ps.tile([128, 1024], F32, tag="dw", bufs=2)
            nc.tensor.transpose(etp[0:C2, 0:C2], ETn[:],
                                id_f32[0:C2, 0:C2])
            nc.scalar.copy(Ebd[:], etp[0:C2, 0:C2])

            # ---------- out2 = attn @ v, both batches per matmul ----------
            for w in range(NW):
                y0 = w * WR
                o2st = stgp.tile([C2, WR * W], FP16, tag="o2st", bufs=8)
                for pr in range(2):
                    pc0 = pr * 1024
                    o2ps = ps.tile([128, 1024], F32,
                                   tag=("cv" if pr == 0 else "dw"), bufs=2)
                    for hf in range(2):
                        nc.tensor.matmul(
                            o2ps[0:C2, hf * 512:(hf + 1) * 512], Ebd[:],
                            vl[0:C2, y0 * W + pc0 + hf * 512:
                               y0 * W + pc0 + (hf + 1) * 512],
                            start=True, stop=True)
                    if pr == 0:
                        nc.scalar.copy(o2st[:, pc0:pc0 + 1024],
                                       o2ps[0:C2, :])
                    else:
                        nc.vector.tensor_copy(o2st[:, pc0:pc0 + 1024],
                                              o2ps[0:C2, :])
                o2st3 = o2st.rearrange("p (r w) -> p r w", w=W)
                nc.gpsimd.dma_start(
                    out[0:BPC, 96:128, y0:y0 + WR, :],
                    o2st3[:, :, :])

        for _ in range(loops):
            one_pass()

    nc.compile()
    return nc


_NC_CACHE = None


def _get_nc():
    global _NC_CACHE
    if _NC_CACHE is None:
        _NC_CACHE = build_nc()
    return _NC_CACHE


def kernel(x, dw_w, dw_b, qkvl_w, qkvl_b):
    x = np.ascontiguousarray(np.asarray(x).astype(np.float16))
    shared = {
        "dw_w": np.ascontiguousarray(np.asarray(dw_w, dtype=np.float32)),
        "dw_b": np.ascontiguousarray(np.asarray(dw_b, dtype=np.float32)),
        "qkvl_w": np.ascontiguousarray(np.asarray(qkvl_w, dtype=np.float32)),
        "qkvl_b": np.ascontiguousarray(np.asarray(qkvl_b, dtype=np.float32)),
    }
    nc = _get_nc()
    in_maps = [
        {"x": x[c * BPC:(c + 1) * BPC], **shared} for c in range(N_CORES)
    ]
    res = bass_utils.run_bass_kernel_spmd(nc, in_maps,
                                          core_ids=list(range(N_CORES)))
    return np.concatenate(
        [np.asarray(res.results[c]["out"]).astype(np.float32)
         for c in range(N_CORES)], axis=0)


# revision 38
# speedup vs baseline: 1.2743x; 1.1365x over previous
"""Trainium2 Bass kernel for the ELGCA block (dwconv3x3+gelu || conv1x1+gelu
-> pooled linear attention), data-parallel over batch on 8 NeuronCores.

Self-contained: hardcodes shapes B=16, C=128, H=W=128, f32 I/O.
kernel(**inputs) takes full unsharded inputs, returns the FULL f32 output.

v8 (per core, BPC=2 local images, partitions p = b*64 + c):
  - fp16 end-to-end on the matmul paths: the host pre-converts x to fp16
    (halves HBM read traffic, removes all on-chip casts); every PE matmul
    runs at 1 cycle/row.  A-side (q|k) precision in fp16 gives ~1.2e-2
    rel err end-to-end (bf16 fails at 0.34: softmax-logit amplification
    needs >=10 mantissa bits).  Pooling / qk / softmax stay f32.
  - dwconv3x3: 7 taps on PE as diagonal fp16 matmuls (tap-major LDW),
    2 taps (dx=1) on DVE in 4x/2x perf mode over the flat padded slab,
    merged via an identity matmul.  dwconv PSUM is half-window double
    buffered so next window's taps never wait on this window's gelu.
  - PE kept at HAM K=8/8: warm-up matmuls at start + through the serial
    softmax stretch; per-window emission order keeps the PE queue fed.
  - DMA: bulk stores (x1, out2) + slab loads on gpsimd SWDGE (spreads
    across all 16 SDMA engines; HWDGE rings degrade to ~2 engines for
    stores), x2 loads on scalar HWDGE, l stores on sync HWDGE.
  - outputs written fp16, widened to f32 on the host.
"""

import numpy as np
from contextlib import ExitStack

import concourse.bass as bass
import concourse.tile as tile
from concourse import bacc, mybir
from concourse import bass_utils
from concourse.masks import make_identity

F32 = mybir.dt.float32
FP16 = mybir.dt.float16
AX = mybir.AxisListType
ALU = mybir.AluOpType
ACTF = mybir.ActivationFunctionType

N_CORES = 8
B_TOT, C, H, W = 16, 128, 128, 128
BPC = B_TOT // N_CORES          # 2 images per core
HW = H * W                      # 16384
C2 = C // 2                     # 64
C4 = C // 4                     # 32
WP = W + 2                      # padded row width (130)
NW = 8                          # number of 16-row windows
WR = H // NW                    # image rows per window (16)
NP = (H // 2) * (W // 2)        # 4096 pooled positions
W2 = W // 2                     # 64
FLAT = 18 * WP                  # flat padded slab size (2340)

# taps: index t = dy*3+dx; PE takes 7, DVE takes 2 (dx=1 keeps the flat
# contiguous offset 4B-aligned for the DVE 2x/4x perf modes)
PE_TAPS = [(0, 0), (0, 1), (0, 2), (1, 0), (1, 2), (2, 0), (2, 2)]


def build_nc(loops=1):
    nc = bacc.Bacc("TRN2", target_bir_lowering=False, debug=False,
                   num_devices=N_CORES)
    x = nc.dram_tensor("x", [BPC, C, H, W], FP16, kind="ExternalInput").ap()
    dw_w = nc.dram_tensor("dw_w", [C2, 1, 3, 3], F32, kind="ExternalInput").ap()
    dw_b = nc.dram_tensor("dw_b", [C2], F32, kind="ExternalInput").ap()
    qw = nc.dram_tensor("qkvl_w", [C, C2, 1, 1], F32, kind="ExternalInput").ap()
    qb = nc.dram_tensor("qkvl_b", [C], F32, kind="ExternalInput").ap()
    out = nc.dram_tensor("out", [BPC, C, H, W], FP16, kind="ExternalOutput").ap()

    with tile.TileContext(nc) as tc, ExitStack() as ctx:
        consts = ctx.enter_context(tc.tile_pool(name="consts", bufs=1))
        inp = ctx.enter_context(tc.tile_pool(name="inp", bufs=4))
        bigp = ctx.enter_context(tc.tile_pool(name="bigp", bufs=1))
        stgp = ctx.enter_context(tc.tile_pool(name="stgp", bufs=2))
        ps = ctx.enter_context(tc.tile_pool(name="ps", bufs=1, space="PSUM"))

        id_f32 = consts.tile([128, 128], F32)
        make_identity(nc, id_f32[:])

        def issue_inputs(w):
            """DMA window w's inputs and return (slab, slab3, x2t)."""
            y0 = w * WR
            ys = max(y0 - 1, 0)
            ye = min(y0 + WR + 1, H)
            nrows = ye - ys
            rs = 0 if w > 0 else 1
            slab = inp.tile([128, FLAT], FP16, tag="slab")
            slab3 = slab.rearrange("p (r w) -> p r w", w=WP)
            nc.vector.memset(slab3[:, :, 0:1], 0.0)
            nc.vector.memset(slab3[:, :, WP - 1:WP], 0.0)
            if w == 0:
                nc.vector.memset(slab3[:, 0:1, :], 0.0)
            if w == NW - 1:
                nc.vector.memset(slab3[:, 17:18, :], 0.0)
            for b in range(BPC):
                nc.gpsimd.dma_start(
                    slab3[C2 * b:C2 * b + C2, rs:rs + nrows, 1:W + 1],
                    x[b:b + 1, 0:C2, ys:ye, :])
            x2t = inp.tile([128, WR * W], FP16, tag="x2t")
            x2t3 = x2t.rearrange("p (r w) -> p r w", w=W)
            for b in range(BPC):
                nc.gpsimd.dma_start(x2t3[C2 * b:C2 * b + C2, :, :],
                                    x[b:b + 1, C2:C, y0:y0 + WR, :])
            return slab, slab3, x2t

        # input DMAs for the first windows go out before the consts
        # chain occupies the queues
        pend = [issue_inputs(0), issue_inputs(1), issue_inputs(2)]

        # warm the PE (HAM throttle) while the first slab DMA is in
        # flight: dead accumulating matmuls on the identity
        wup = ps.tile([128, 1024], F32, tag="cv", bufs=2)
        for i in range(20):
            nc.tensor.matmul(wup[:, 0:128], id_f32[:], id_f32[:],
                             start=(i == 0), stop=(i == 19))

        # ---------------- constants (sync-queue loads) ----------------
        w_tile = consts.tile([128, 9], F32)
        dw9 = dw_w.rearrange("c o kh kw -> c (o kh kw)")
        nc.sync.dma_start(w_tile[0:C2, :], dw9)
        nc.sync.dma_start(w_tile[C2:128, :], dw9)

        # qkvl_w: load [128oc, 64ic] contiguous, PE-transpose to [64ic, 128oc]
        qw_oc = consts.tile([128, C2], F32)
        nc.sync.dma_start(qw_oc[:], qw.rearrange("o i kh kw -> o (i kh kw)"))
        qwT_ps = ps.tile([128, 1024], F32, tag="cv", bufs=2)
        nc.tensor.transpose(qwT_ps[0:C2, 0:128], qw_oc[:], id_f32[:])
        qwT_sb = consts.tile([C2, 128], F32)
        nc.scalar.copy(qwT_sb[:], qwT_ps[0:C2, 0:128])

        # row-vector loads (1 descriptor each) for biases
        qb_row = consts.tile([1, C], F32)
        nc.sync.dma_start(qb_row[:], qb.unsqueeze(0))
        dwb_row = consts.tile([1, C2], F32)
        nc.sync.dma_start(dwb_row[:], dw_b.unsqueeze(0))

        # permuted bias rows -> PE transpose to per-partition columns
        # brow col-blocks: biasA = [qb0:32|qb0:32|qb32:64|qb32:64],
        # biasB = [qb64:96|...], dwb = [dwb|dwb]
        brow = consts.tile([1, 3 * 128], F32)
        nc.scalar.copy(brow[:, 0:C4], qb_row[:, 0:C4])
        nc.scalar.copy(brow[:, C4:C2], qb_row[:, 0:C4])
        nc.scalar.copy(brow[:, C2:96], qb_row[:, C4:C2])
        nc.scalar.copy(brow[:, 96:128], qb_row[:, C4:C2])
        nc.scalar.copy(brow[:, 128:160], qb_row[:, C2:96])
        nc.scalar.copy(brow[:, 160:192], qb_row[:, C2:96])
        nc.scalar.copy(brow[:, 192:224], qb_row[:, 96:128])
        nc.scalar.copy(brow[:, 224:256], qb_row[:, 96:128])
        nc.scalar.copy(brow[:, 256:320], dwb_row[:, 0:C2])
        nc.scalar.copy(brow[:, 320:384], dwb_row[:, 0:C2])
        bcol_ps = ps.tile([128, 1024], F32, tag="dw", bufs=2)
        for i in range(3):
            nc.tensor.transpose(bcol_ps[:, i:i + 1],
                                brow[0:1, i * 128:(i + 1) * 128],
                                id_f32[0:1, 0:1])
        bcol = consts.tile([128, 3], F32)
        nc.scalar.copy(bcol[:], bcol_ps[:, 0:3])
        biasA = bcol[:, 0:1]
        biasB = bcol[:, 1:2]
        dwb_t = bcol[:, 2:3]

        # 9 diagonal tap matrices, fp16 (direct f32->fp16 on DVE)
        wdiag = consts.tile([128, 9 * 128], FP16)
        for t in range(9):
            nc.vector.tensor_scalar_mul(
                wdiag[:, t * 128:(t + 1) * 128], id_f32[:],
                w_tile[:, t:t + 1])

        # conv1x1 block-diagonal weights from qwT_sb (direct fp16 copies)
        lhsAB = consts.tile([128, 256], FP16)
        nc.vector.memset(lhsAB[:], 0.0)
        nc.scalar.copy(lhsAB[0:C2, 0:C4], qwT_sb[:, 0:C4])
        nc.scalar.copy(lhsAB[C2:128, C4:C2], qwT_sb[:, 0:C4])
        nc.scalar.copy(lhsAB[0:C2, C2:96], qwT_sb[:, C4:C2])
        nc.scalar.copy(lhsAB[C2:128, 96:128], qwT_sb[:, C4:C2])
        nc.scalar.copy(lhsAB[0:C2, 128:160], qwT_sb[:, C2:96])
        nc.scalar.copy(lhsAB[C2:128, 160:192], qwT_sb[:, C2:96])
        nc.scalar.copy(lhsAB[0:C2, 192:224], qwT_sb[:, 96:128])
        nc.scalar.copy(lhsAB[C2:128, 224:256], qwT_sb[:, 96:128])
        id_h = consts.tile([128, 128], FP16)
        nc.vector.tensor_copy(id_h[:], id_f32[:])
        lhsA = lhsAB[:, 0:128]
        lhsB = lhsAB[:, 128:256]

        def one_pass():
            # persistent per-pass buffers
            vl = bigp.tile([128, HW], FP16, tag="vl")   # v rows 0:64, l 64:128
            hp = bigp.tile([128, H * W2], F32, tag="hp")
            hp3 = hp.rearrange("p (r w) -> p r w", w=W2)
            hpv = hp.rearrange("p (o two w) -> p o two w", two=2, w=W2)
            pp = bigp.tile([128, NP], F32, tag="pp")
            pp3 = pp.rearrange("p (r w) -> p r w", w=W2)
            qk_acc = bigp.tile([C2, C2], F32, tag="qk")

            def attn_pp(sw):
                """v-pool pooled rows [8sw, 8sw+8) (DVE part of attn)."""
                o0 = 8 * sw
                nc.vector.tensor_add(pp3[0:C2, o0:o0 + 8, :],
                                     hpv[0:C2, o0:o0 + 8, 0, :],
                                     hpv[0:C2, o0:o0 + 8, 1, :])
                lo = max(o0, 1)
                nc.vector.tensor_add(pp3[0:C2, lo:o0 + 8, :],
                                     pp3[0:C2, lo:o0 + 8, :],
                                     hpv[0:C2, lo - 1:o0 + 7, 1, :])
                nc.vector.tensor_max(pp3[C2:128, o0:o0 + 8, :],
                                     hpv[C2:128, o0:o0 + 8, 0, :],
                                     hpv[C2:128, o0:o0 + 8, 1, :])

            def attn_tr(sw):
                """transpose the 4 new 128-position chunks."""
                trps = ps.tile([128, 1024], F32, tag="cv", bufs=2)
                for jj in range(4):
                    ch = 4 * sw + jj
                    nc.tensor.transpose(trps[:, jj * 128:(jj + 1) * 128],
                                        pp[:, ch * 128:(ch + 1) * 128],
                                        id_f32[:])
                trsb = stgp.tile([128, 512], FP16, tag="trsb")
                nc.scalar.copy(trsb[:], trps[:, 0:512])
                return trsb

            def attn_qk(sw, trsb):
                """accumulate this slice's qk partial."""
                qkps = ps.tile([128, 1024], F32, tag="cv", bufs=2)
                for jj in range(4):
                    nc.tensor.matmul(
                        qkps[0:C2, 0:C2],
                        trsb[:, jj * 128 + C2:(jj + 1) * 128],
                        trsb[:, jj * 128:jj * 128 + C2],
                        start=(jj == 0), stop=(jj == 3))
                if sw == 0:
                    nc.scalar.copy(qk_acc[:], qkps[0:C2, 0:C2])
                else:
                    qktmp = stgp.tile([C2, C2], F32, tag="qktmp")
                    nc.scalar.copy(qktmp[:], qkps[0:C2, 0:C2])
                    nc.gpsimd.tensor_add(qk_acc[:], qk_acc[:], qktmp[:])

            for w in range(NW):
                y0 = w * WR
                slab, slab3, x2t = pend.pop(0)
                if w + 3 < NW:
                    pend.append(issue_inputs(w + 3))

                qg = stgp.tile([128, WR * W], F32, tag="qg", bufs=1)
                x1st = stgp.tile([128, WR * W], FP16, tag="x1st")

                # ---- PE dwconv taps first: keeps the PE streaming while
                # DVE computes the dx=1 taps and the attn pooling.
                # Tap-major across both halves: one LDWEIGHTS per tap. ----
                dwps = []
                for _h in range(2):
                    dwtile = ps.tile([128, 1024], F32, tag="dw", bufs=2)
                    dwps.append(dwtile)
                for ti, (dy, dx) in enumerate(PE_TAPS):
                    t = dy * 3 + dx
                    for h in range(2):
                        for qq in range(2):
                            la = h * 8 + qq * 4
                            nc.tensor.matmul(
                                dwps[h][:, qq * 512:(qq + 1) * 512],
                                wdiag[:, t * 128:(t + 1) * 128],
                                slab3[:, la + dy:la + dy + 4, dx:dx + W],
                                start=(ti == 0), stop=False)

                # ---- attn pooling + transposes for the previous window ----
                trsb = None
                if w >= 1:
                    attn_pp(w - 1)
                    trsb = attn_tr(w - 1)

                # ---- dx=1 dwconv taps on DVE (flat contiguous, 4x/2x) ----
                dwacc = stgp.tile([128, 16 * WP], FP16, tag="dwacc", bufs=1)
                dwtmp = stgp.tile([128, 16 * WP], FP16, tag="dwtmp", bufs=1)
                nc.vector.tensor_scalar_mul(
                    dwacc[:, 0:16 * WP], slab[:, WP:WP + 16 * WP],
                    w_tile[:, 4:5])
                nc.vector.tensor_scalar_mul(
                    dwtmp[:, 0:16 * WP], slab[:, 2 * WP:2 * WP + 16 * WP],
                    w_tile[:, 7:8])
                nc.vector.tensor_add(dwacc[:, 0:16 * WP],
                                     dwacc[:, 0:16 * WP],
                                     dwtmp[:, 0:16 * WP])
                dwacc3 = dwacc.rearrange("p (r w) -> p r w", w=WP)

                # ---- conv1x1 A pairs (fp16 -> f32 qg) ----
                for pr in range(2):
                    pc0 = pr * 1024
                    Aps = ps.tile([128, 1024], F32, tag="cv", bufs=2)
                    for hf in range(2):
                        nc.tensor.matmul(
                            Aps[:, hf * 512:(hf + 1) * 512], lhsA,
                            x2t[:, pc0 + hf * 512:pc0 + (hf + 1) * 512],
                            start=True, stop=True)
                    nc.scalar.activation(qg[:, pc0:pc0 + 1024], Aps[:],
                                         ACTF.Gelu, bias=biasA)

                # ---- attn qk for the previous window ----
                if w >= 1:
                    attn_qk(w - 1, trsb)

                # ---- conv1x1 B pairs -> vl ----
                for pr in range(2):
                    pc0 = pr * 1024
                    Bps = ps.tile([128, 1024], F32, tag="cv", bufs=2)
                    for hf in range(2):
                        nc.tensor.matmul(
                            Bps[:, hf * 512:(hf + 1) * 512], lhsB,
                            x2t[:, pc0 + hf * 512:pc0 + (hf + 1) * 512],
                            start=True, stop=True)
                    nc.scalar.activation(vl[:, y0 * W + pc0:
                                            y0 * W + pc0 + 1024],
                                         Bps[:], ACTF.Gelu, bias=biasB)

                # ---- merge DVE taps into the PE accumulators, gelu ----
                for h in range(2):
                    dw = dwps[h]
                    for qq in range(2):
                        la = h * 8 + qq * 4
                        nc.tensor.matmul(
                            dw[:, qq * 512:(qq + 1) * 512], id_h[:],
                            dwacc3[:, la:la + 4, 1:W + 1],
                            start=False, stop=True)
                    nc.scalar.activation(x1st[:, h * 1024:(h + 1) * 1024],
                                         dw[:], ACTF.Gelu, bias=dwb_t)
                x1st3 = x1st.rearrange("p (r w) -> p r w", w=W)
                for b in range(BPC):
                    nc.scalar.dma_start(out[b:b + 1, 0:C2, y0:y0 + WR, :],
                                        x1st3[C2 * b:C2 * b + C2, :, :])

                # ---- l output DMA for this window ----
                nc.gpsimd.dma_start(
                    out[0:BPC, C2:96, y0:y0 + WR, :],
                    vl[C2:128, y0 * W:(y0 + WR) * W]
                    .rearrange("p (r w) -> p r w", w=W))

                # ---- horizontal pooling for this window ----
                qg3 = qg.rearrange("p (r w2 two) -> p r w2 two", two=2, w2=W2)
                nc.vector.tensor_add(hp3[0:C2, y0:y0 + WR, :],
                                     qg3[0:C2, :, :, 0], qg3[0:C2, :, :, 1])
                nc.vector.tensor_add(hp3[0:C2, y0:y0 + WR, 1:W2],
                                     hp3[0:C2, y0:y0 + WR, 1:W2],
                                     qg3[0:C2, :, 0:W2 - 1, 1])
                nc.vector.tensor_max(hp3[C2:128, y0:y0 + WR, :],
                                     qg3[C2:128, :, :, 0],
                                     qg3[C2:128, :, :, 1])

            attn_pp(NW - 1)
            attn_qk(NW - 1, attn_tr(NW - 1))

            # keep the PE (HAM) warm through the serial softmax stretch:
            # dead accumulating matmuls, emitted BEFORE the Ebd transpose
            # so they sit ahead of it in the PE's in-order queue
            warmps = ps.tile([128, 1024], F32, tag="cv", bufs=2)
            for i in range(24):
                nc.tensor.matmul(warmps[:, 0:512], id_h[:],
                                 vl[:, i * 512:(i + 1) * 512],
                                 start=(i == 0), stop=(i == 23))

            # ---------- softmax stats -> block-diag attention ----------
            # Both image blocks in one pass: mask off-diagonal blocks to
            # -inf so the row reduce/exp/sum ignore them (exp -> 0, which
            # also zeroes the off-blocks of Ebd for free).
            Ebd = bigp.tile([C2, C2], FP16, tag="Ebd")
            qk9 = bigp.tile([C2, C2], F32, tag="qk9")
            nc.scalar.mul(qk9[:], qk_acc[:], 1.0 / 9.0)
            nc.vector.memset(qk9[0:C4, C4:C2], -1e30)
            nc.vector.memset(qk9[C4:C2, 0:C4], -1e30)
            nmax = bigp.tile([C2, 1], F32, tag="nmax")
            nc.vector.tensor_reduce(nmax[:], qk9[:], axis=AX.X,
                                    op=ALU.max, negate=True)
            ET = bigp.tile([C2, C2], F32, tag="ET")
            nc.scalar.activation(ET[:], qk9[:], ACTF.Exp,
                                 bias=nmax[:, 0:1])
            ssum = bigp.tile([C2, 1], F32, tag="ssum")
            nc.vector.reduce_sum(ssum[:], ET[:], axis=AX.X)
            rec = bigp.tile([C2, 1], F32, tag="rec")
            nc.vector.reciprocal(rec[:], ssum[:])
            ETn = bigp.tile([C2, C2], F32, tag="ETn")
            nc.vector.tensor_scalar_mul(ETn[:], ET[:], rec[:, 0:1])
            etp = ps.tile([128, 1024], F32, tag="dw", bufs=2)
            nc.tensor.transpose(etp[0:C2, 0:C2], ETn[:],
                                id_f32[0:C2, 0:C2])
            nc.scalar.copy(Ebd[:], etp[0:C2, 0:C2])

            # ---------- out2 = attn @ v, both batches per matmul ----------
            for w in range(NW):
                y0 = w * WR
                o2st = stgp.tile([C2, WR * W], FP16, tag="o2st", bufs=8)
                for pr in range(2):
                    pc0 = pr * 1024
                    o2ps = ps.tile([128, 1024], F32,
                                   tag=("cv" if pr == 0 else "dw"), bufs=2)
                    for hf in range(2):
                        nc.tensor.matmul(
                            o2ps[0:C2, hf * 512:(hf + 1) * 512], Ebd[:],
                            vl[0:C2, y0 * W + pc0 + hf * 512:
                               y0 * W + pc0 + (hf + 1) * 512],
                            start=True, stop=True)
                    if pr == 0:
                        nc.scalar.copy(o2st[:, pc0:pc0 + 1024],
                                       o2ps[0:C2, :])
                    else:
                        nc.vector.tensor_copy(o2st[:, pc0:pc0 + 1024],
                                              o2ps[0:C2, :])
                o2st3 = o2st.rearrange("p (r w) -> p r w", w=W)
                nc.gpsimd.dma_start(
                    out[0:BPC, 96:128, y0:y0 + WR, :],
                    o2st3[:, :, :])

        for _ in range(loops):
            one_pass()

    nc.compile()
    return nc


_NC_CACHE = None


def _get_nc():
    global _NC_CACHE
    if _NC_CACHE is None:
        _NC_CACHE = build_nc()
    return _NC_CACHE


def kernel(x, dw_w, dw_b, qkvl_w, qkvl_b):
    x = np.ascontiguousarray(np.asarray(x).astype(np.float16))
    shared = {
        "dw_w": np.ascontiguousarray(np.asarray(dw_w, dtype=np.float32)),
        "dw_b": np.ascontiguousarray(np.asarray(dw_b, dtype=np.float32)),
        "qkvl_w": np.ascontiguousarray(np.asarray(qkvl_w, dtype=np.float32)),
        "qkvl_b": np.ascontiguousarray(np.asarray(qkvl_b, dtype=np.float32)),
    }
    nc = _get_nc()
    in_maps = [
        {"x": x[c * BPC:(c + 1) * BPC], **shared} for c in range(N_CORES)
    ]
    res = bass_utils.run_bass_kernel_spmd(nc, in_maps,
                                          core_ids=list(range(N_CORES)))
    return np.concatenate(
        [np.asarray(res.results[c]["out"]).astype(np.float32)
         for c in range(N_CORES)], axis=0)


# revision 40
# speedup vs baseline: 1.4099x; 1.1064x over previous
"""Trainium2 Bass kernel for the ELGCA block (dwconv3x3+gelu || conv1x1+gelu
-> pooled linear attention), data-parallel over batch on 8 NeuronCores.

Self-contained: hardcodes shapes B=16, C=128, H=W=128, f32 I/O.
kernel(**inputs) takes full unsharded inputs, returns the FULL f32 output.

v8 (per core, BPC=2 local images, partitions p = b*64 + c):
  - fp16 end-to-end on the matmul paths: the host pre-converts x to fp16
    (halves HBM read traffic, removes all on-chip casts); every PE matmul
    runs at 1 cycle/row.  A-side (q|k) precision in fp16 gives ~1.2e-2
    rel err end-to-end (bf16 fails at 0.34: softmax-logit amplification
    needs >=10 mantissa bits).  Pooling / qk / softmax stay f32.
  - dwconv3x3: 7 taps on PE as diagonal fp16 matmuls (tap-major LDW),
    2 taps (dx=1) on DVE in 4x/2x perf mode over the flat padded slab,
    merged via an identity matmul.  dwconv PSUM is half-window double
    buffered so next window's taps never wait on this window's gelu.
  - PE kept at HAM K=8/8: warm-up matmuls at start + through the serial
    softmax stretch; per-window emission order keeps the PE queue fed.
  - DMA: bulk stores (x1, out2) + slab loads on gpsimd SWDGE (spreads
    across all 16 SDMA engines; HWDGE rings degrade to ~2 engines for
    stores), x2 loads on scalar HWDGE, l stores on sync HWDGE.
  - outputs written fp16, widened to f32 on the host.
"""

import numpy as np
from contextlib import ExitStack

import concourse.bass as bass
import concourse.tile as tile
from concourse import bacc, mybir
from concourse import bass_utils
from concourse.masks import make_identity

F32 = mybir.dt.float32
FP16 = mybir.dt.float16
AX = mybir.AxisListType
ALU = mybir.AluOpType
ACTF = mybir.ActivationFunctionType

N_CORES = 8
B_TOT, C, H, W = 16, 128, 128, 128
BPC = B_TOT // N_CORES          # 2 images per core
HW = H * W                      # 16384
C2 = C // 2                     # 64
C4 = C // 4                     # 32
WP = W + 2                      # padded row width (130)
NW = 8                          # number of 16-row windows
WR = H // NW                    # image rows per window (16)
NP = (H // 2) * (W // 2)        # 4096 pooled positions
W2 = W // 2                     # 64
FLAT = 18 * WP                  # flat padded slab size (2340)

# taps: index t = dy*3+dx; PE takes 7, DVE takes 2 (dx=1 keeps the flat
# contiguous offset 4B-aligned for the DVE 2x/4x perf modes)
PE_TAPS = [(0, 0), (0, 1), (0, 2), (1, 0), (1, 2), (2, 0), (2, 2)]


def build_nc(loops=1):
    nc = bacc.Bacc("TRN2", target_bir_lowering=False, debug=False,
                   num_devices=N_CORES)
    x = nc.dram_tensor("x", [BPC, C, H, W], FP16, kind="ExternalInput").ap()
    dw_w = nc.dram_tensor("dw_w", [C2, 1, 3, 3], F32, kind="ExternalInput").ap()
    dw_b = nc.dram_tensor("dw_b", [C2], F32, kind="ExternalInput").ap()
    qw = nc.dram_tensor("qkvl_w", [C, C2, 1, 1], F32, kind="ExternalInput").ap()
    qb = nc.dram_tensor("qkvl_b", [C], F32, kind="ExternalInput").ap()
    out = nc.dram_tensor("out", [BPC, C, H, W], FP16, kind="ExternalOutput").ap()

    with tile.TileContext(nc) as tc, ExitStack() as ctx:
        consts = ctx.enter_context(tc.tile_pool(name="consts", bufs=1))
        inp = ctx.enter_context(tc.tile_pool(name="inp", bufs=4))
        bigp = ctx.enter_context(tc.tile_pool(name="bigp", bufs=1))
        stgp = ctx.enter_context(tc.tile_pool(name="stgp", bufs=2))
        ps = ctx.enter_context(tc.tile_pool(name="ps", bufs=1, space="PSUM"))

        id_f32 = consts.tile([128, 128], F32)
        make_identity(nc, id_f32[:])

        def issue_inputs(w):
            """DMA window w's inputs and return (slab, slab3, x2t)."""
            y0 = w * WR
            ys = max(y0 - 1, 0)
            ye = min(y0 + WR + 1, H)
            nrows = ye - ys
            rs = 0 if w > 0 else 1
            slab = inp.tile([128, FLAT], FP16, tag="slab")
            slab3 = slab.rearrange("p (r w) -> p r w", w=WP)
            nc.vector.memset(slab3[:, :, 0:1], 0.0)
            nc.vector.memset(slab3[:, :, WP - 1:WP], 0.0)
            if w == 0:
                nc.vector.memset(slab3[:, 0:1, :], 0.0)
            if w == NW - 1:
                nc.vector.memset(slab3[:, 17:18, :], 0.0)
            for b in range(BPC):
                nc.gpsimd.dma_start(
                    slab3[C2 * b:C2 * b + C2, rs:rs + nrows, 1:W + 1],
                    x[b:b + 1, 0:C2, ys:ye, :])
            x2t = inp.tile([128, WR * W], FP16, tag="x2t")
            x2t3 = x2t.rearrange("p (r w) -> p r w", w=W)
            for b in range(BPC):
                nc.gpsimd.dma_start(x2t3[C2 * b:C2 * b + C2, :, :],
                                    x[b:b + 1, C2:C, y0:y0 + WR, :])
            return slab, slab3, x2t

        # input DMAs for the first windows go out before the consts
        # chain occupies the queues
        pend = [issue_inputs(0), issue_inputs(1), issue_inputs(2)]

        # warm the PE (HAM throttle) while the first slab DMA is in
        # flight: dead accumulating matmuls on the identity
        wup = ps.tile([128, 1024], F32, tag="cv", bufs=2)
        for i in range(20):
            nc.tensor.matmul(wup[:, 0:128], id_f32[:], id_f32[:],
                             start=(i == 0), stop=(i == 19))

        # ---------------- constants (sync-queue loads) ----------------
        w_tile = consts.tile([128, 9], F32)
        dw9 = dw_w.rearrange("c o kh kw -> c (o kh kw)")
        nc.sync.dma_start(w_tile[0:C2, :], dw9)
        nc.sync.dma_start(w_tile[C2:128, :], dw9)

        # qkvl_w: load [128oc, 64ic] contiguous, PE-transpose to [64ic, 128oc]
        qw_oc = consts.tile([128, C2], F32)
        nc.sync.dma_start(qw_oc[:], qw.rearrange("o i kh kw -> o (i kh kw)"))
        qwT_ps = ps.tile([128, 1024], F32, tag="cv", bufs=2)
        nc.tensor.transpose(qwT_ps[0:C2, 0:128], qw_oc[:], id_f32[:])
        qwT_sb = consts.tile([C2, 128], F32)
        nc.scalar.copy(qwT_sb[:], qwT_ps[0:C2, 0:128])

        # row-vector loads (1 descriptor each) for biases
        qb_row = consts.tile([1, C], F32)
        nc.sync.dma_start(qb_row[:], qb.unsqueeze(0))
        dwb_row = consts.tile([1, C2], F32)
        nc.sync.dma_start(dwb_row[:], dw_b.unsqueeze(0))

        # permuted bias rows -> PE transpose to per-partition columns
        # brow col-blocks: biasA = [qb0:32|qb0:32|qb32:64|qb32:64],
        # biasB = [qb64:96|...], dwb = [dwb|dwb]
        brow = consts.tile([1, 3 * 128], F32)
        nc.scalar.copy(brow[:, 0:C4], qb_row[:, 0:C4])
        nc.scalar.copy(brow[:, C4:C2], qb_row[:, 0:C4])
        nc.scalar.copy(brow[:, C2:96], qb_row[:, C4:C2])
        nc.scalar.copy(brow[:, 96:128], qb_row[:, C4:C2])
        nc.scalar.copy(brow[:, 128:160], qb_row[:, C2:96])
        nc.scalar.copy(brow[:, 160:192], qb_row[:, C2:96])
        nc.scalar.copy(brow[:, 192:224], qb_row[:, 96:128])
        nc.scalar.copy(brow[:, 224:256], qb_row[:, 96:128])
        nc.scalar.copy(brow[:, 256:320], dwb_row[:, 0:C2])
        nc.scalar.copy(brow[:, 320:384], dwb_row[:, 0:C2])
        bcol_ps = ps.tile([128, 1024], F32, tag="dw", bufs=2)
        for i in range(3):
            nc.tensor.transpose(bcol_ps[:, i:i + 1],
                                brow[0:1, i * 128:(i + 1) * 128],
                                id_f32[0:1, 0:1])
        bcol = consts.tile([128, 3], F32)
        nc.scalar.copy(bcol[:], bcol_ps[:, 0:3])
        biasA = bcol[:, 0:1]
        biasB = bcol[:, 1:2]
        dwb_t = bcol[:, 2:3]

        # 9 diagonal tap matrices, fp16 (direct f32->fp16 on DVE)
        wdiag = consts.tile([128, 9 * 128], FP16)
        for t in range(9):
            nc.vector.tensor_scalar_mul(
                wdiag[:, t * 128:(t + 1) * 128], id_f32[:],
                w_tile[:, t:t + 1])

        # conv1x1 block-diagonal weights from qwT_sb (direct fp16 copies)
        lhsAB = consts.tile([128, 256], FP16)
        nc.vector.memset(lhsAB[:], 0.0)
        nc.scalar.copy(lhsAB[0:C2, 0:C4], qwT_sb[:, 0:C4])
        nc.scalar.copy(lhsAB[C2:128, C4:C2], qwT_sb[:, 0:C4])
        nc.scalar.copy(lhsAB[0:C2, C2:96], qwT_sb[:, C4:C2])
        nc.scalar.copy(lhsAB[C2:128, 96:128], qwT_sb[:, C4:C2])
        nc.scalar.copy(lhsAB[0:C2, 128:160], qwT_sb[:, C2:96])
        nc.scalar.copy(lhsAB[C2:128, 160:192], qwT_sb[:, C2:96])
        nc.scalar.copy(lhsAB[0:C2, 192:224], qwT_sb[:, 96:128])
        nc.scalar.copy(lhsAB[C2:128, 224:256], qwT_sb[:, 96:128])
        id_h = consts.tile([128, 128], FP16)
        nc.vector.tensor_copy(id_h[:], id_f32[:])
        lhsA = lhsAB[:, 0:128]
        lhsB = lhsAB[:, 128:256]

        def one_pass():
            # persistent per-pass buffers
            vl = bigp.tile([128, HW], FP16, tag="vl")   # v rows 0:64, l 64:128
            hp = bigp.tile([128, H * W2], F32, tag="hp")
            hp3 = hp.rearrange("p (r w) -> p r w", w=W2)
            hpv = hp.rearrange("p (o two w) -> p o two w", two=2, w=W2)
            pp = bigp.tile([128, NP], F32, tag="pp")
            pp3 = pp.rearrange("p (r w) -> p r w", w=W2)
            qk_acc = bigp.tile([C2, C2], F32, tag="qk")

            def attn_pp(sw):
                """v-pool pooled rows [8sw, 8sw+8) (DVE part of attn)."""
                o0 = 8 * sw
                nc.vector.tensor_add(pp3[0:C2, o0:o0 + 8, :],
                                     hpv[0:C2, o0:o0 + 8, 0, :],
                                     hpv[0:C2, o0:o0 + 8, 1, :])
                lo = max(o0, 1)
                nc.vector.tensor_add(pp3[0:C2, lo:o0 + 8, :],
                                     pp3[0:C2, lo:o0 + 8, :],
                                     hpv[0:C2, lo - 1:o0 + 7, 1, :])
                nc.vector.tensor_max(pp3[C2:128, o0:o0 + 8, :],
                                     hpv[C2:128, o0:o0 + 8, 0, :],
                                     hpv[C2:128, o0:o0 + 8, 1, :])

            def attn_tr(sw):
                """transpose the 4 new 128-position chunks."""
                trps = ps.tile([128, 1024], F32, tag="cv", bufs=2)
                for jj in range(4):
                    ch = 4 * sw + jj
                    nc.tensor.transpose(trps[:, jj * 128:(jj + 1) * 128],
                                        pp[:, ch * 128:(ch + 1) * 128],
                                        id_f32[:])
                trsb = stgp.tile([128, 512], FP16, tag="trsb")
                nc.scalar.copy(trsb[:], trps[:, 0:512])
                return trsb

            def attn_qk(sw, trsb):
                """accumulate this slice's qk partial."""
                qkps = ps.tile([128, 1024], F32, tag="cv", bufs=2)
                for jj in range(4):
                    nc.tensor.matmul(
                        qkps[0:C2, 0:C2],
                        trsb[:, jj * 128 + C2:(jj + 1) * 128],
                        trsb[:, jj * 128:jj * 128 + C2],
                        start=(jj == 0), stop=(jj == 3))
                if sw == 0:
                    nc.scalar.copy(qk_acc[:], qkps[0:C2, 0:C2])
                else:
                    qktmp = stgp.tile([C2, C2], F32, tag="qktmp")
                    nc.scalar.copy(qktmp[:], qkps[0:C2, 0:C2])
                    nc.gpsimd.tensor_add(qk_acc[:], qk_acc[:], qktmp[:])

            for w in range(NW):
                y0 = w * WR
                slab, slab3, x2t = pend.pop(0)
                if w + 3 < NW:
                    pend.append(issue_inputs(w + 3))

                qg = stgp.tile([128, WR * W], F32, tag="qg", bufs=1)
                x1st = stgp.tile([128, WR * W], FP16, tag="x1st")

                # ---- PE dwconv taps first: keeps the PE streaming while
                # DVE computes the dx=1 taps and the attn pooling.
                # Tap-major across both halves: one LDWEIGHTS per tap. ----
                dwps = []
                for _h in range(2):
                    dwtile = ps.tile([128, 1024], F32, tag="dw", bufs=2)
                    dwps.append(dwtile)
                for ti, (dy, dx) in enumerate(PE_TAPS):
                    t = dy * 3 + dx
                    for h in range(2):
                        for qq in range(2):
                            la = h * 8 + qq * 4
                            nc.tensor.matmul(
                                dwps[h][:, qq * 512:(qq + 1) * 512],
                                wdiag[:, t * 128:(t + 1) * 128],
                                slab3[:, la + dy:la + dy + 4, dx:dx + W],
                                start=(ti == 0), stop=False)

                # ---- attn pooling + transposes for the previous window ----
                trsb = None
                if w >= 1:
                    attn_pp(w - 1)
                    trsb = attn_tr(w - 1)

                # ---- dx=1 dwconv taps on DVE (flat contiguous, 4x/2x) ----
                dwacc = stgp.tile([128, 16 * WP], FP16, tag="dwacc", bufs=1)
                dwtmp = stgp.tile([128, 16 * WP], FP16, tag="dwtmp", bufs=1)
                nc.vector.tensor_scalar_mul(
                    dwacc[:, 0:16 * WP], slab[:, WP:WP + 16 * WP],
                    w_tile[:, 4:5])
                nc.vector.tensor_scalar_mul(
                    dwtmp[:, 0:16 * WP], slab[:, 2 * WP:2 * WP + 16 * WP],
                    w_tile[:, 7:8])
                nc.vector.tensor_add(dwacc[:, 0:16 * WP],
                                     dwacc[:, 0:16 * WP],
                                     dwtmp[:, 0:16 * WP])
                dwacc3 = dwacc.rearrange("p (r w) -> p r w", w=WP)

                # ---- conv1x1 A pairs (fp16 -> f32 qg) ----
                for pr in range(2):
                    pc0 = pr * 1024
                    Aps = ps.tile([128, 1024], F32, tag="cv", bufs=2)
                    for hf in range(2):
                        nc.tensor.matmul(
                            Aps[:, hf * 512:(hf + 1) * 512], lhsA,
                            x2t[:, pc0 + hf * 512:pc0 + (hf + 1) * 512],
                            start=True, stop=True)
                    nc.scalar.activation(qg[:, pc0:pc0 + 1024], Aps[:],
                                         ACTF.Gelu, bias=biasA)

                # ---- attn qk for the previous window ----
                if w >= 1:
                    attn_qk(w - 1, trsb)

                # ---- conv1x1 B pairs -> vl ----
                for pr in range(2):
                    pc0 = pr * 1024
                    Bps = ps.tile([128, 1024], F32, tag="cv", bufs=2)
                    for hf in range(2):
                        nc.tensor.matmul(
                            Bps[:, hf * 512:(hf + 1) * 512], lhsB,
                            x2t[:, pc0 + hf * 512:pc0 + (hf + 1) * 512],
                            start=True, stop=True)
                    nc.scalar.activation(vl[:, y0 * W + pc0:
                                            y0 * W + pc0 + 1024],
                                         Bps[:], ACTF.Gelu, bias=biasB)

                # ---- merge DVE taps into the PE accumulators, gelu ----
                for h in range(2):
                    dw = dwps[h]
                    for qq in range(2):
                        la = h * 8 + qq * 4
                        nc.tensor.matmul(
                            dw[:, qq * 512:(qq + 1) * 512], id_h[:],
                            dwacc3[:, la:la + 4, 1:W + 1],
                            start=False, stop=True)
                    nc.scalar.activation(x1st[:, h * 1024:(h + 1) * 1024],
                                         dw[:], ACTF.Gelu, bias=dwb_t)
                x1st3 = x1st.rearrange("p (r w) -> p r w", w=W)
                for b in range(BPC):
                    nc.scalar.dma_start(out[b:b + 1, 0:C2, y0:y0 + WR, :],
                                        x1st3[C2 * b:C2 * b + C2, :, :])

                # ---- l output DMA for this window ----
                vl3 = vl[C2:128, y0 * W:(y0 + WR) * W] \
                    .rearrange("p (r w) -> p r w", w=W)
                for b in range(BPC):
                    nc.gpsimd.dma_start(
                        out[b:b + 1, C2:96, y0:y0 + WR, :],
                        vl3[C4 * b:C4 * b + C4, :, :])

                # ---- horizontal pooling for this window ----
                qg3 = qg.rearrange("p (r w2 two) -> p r w2 two", two=2, w2=W2)
                nc.vector.tensor_add(hp3[0:C2, y0:y0 + WR, :],
                                     qg3[0:C2, :, :, 0], qg3[0:C2, :, :, 1])
                nc.vector.tensor_add(hp3[0:C2, y0:y0 + WR, 1:W2],
                                     hp3[0:C2, y0:y0 + WR, 1:W2],
                                     qg3[0:C2, :, 0:W2 - 1, 1])
                nc.vector.tensor_max(hp3[C2:128, y0:y0 + WR, :],
                                     qg3[C2:128, :, :, 0],
                                     qg3[C2:128, :, :, 1])

            attn_pp(NW - 1)
            attn_qk(NW - 1, attn_tr(NW - 1))

            # keep the PE (HAM) warm through the serial softmax stretch:
            # dead accumulating matmuls, emitted BEFORE the Ebd transpose
            # so they sit ahead of it in the PE's in-order queue
            warmps = ps.tile([128, 1024], F32, tag="cv", bufs=2)
            for i in range(24):
                nc.tensor.matmul(warmps[:, 0:512], id_h[:],
                                 vl[:, i * 512:(i + 1) * 512],
                                 start=(i == 0), stop=(i == 23))

            # ---------- softmax stats -> block-diag attention ----------
            # Both image blocks in one pass: mask off-diagonal blocks to
            # -inf so the row reduce/exp/sum ignore them (exp -> 0, which
            # also zeroes the off-blocks of Ebd for free).
            Ebd = bigp.tile([C2, C2], FP16, tag="Ebd")
            qk9 = bigp.tile([C2, C2], F32, tag="qk9")
            nc.scalar.mul(qk9[:], qk_acc[:], 1.0 / 9.0)
            nc.vector.memset(qk9[0:C4, C4:C2], -1e30)
            nc.vector.memset(qk9[C4:C2, 0:C4], -1e30)
            nmax = bigp.tile([C2, 1], F32, tag="nmax")
            nc.vector.tensor_reduce(nmax[:], qk9[:], axis=AX.X,
                                    op=ALU.max, negate=True)
            ET = bigp.tile([C2, C2], F32, tag="ET")
            nc.scalar.activation(ET[:], qk9[:], ACTF.Exp,
                                 bias=nmax[:, 0:1])
            ssum = bigp.tile([C2, 1], F32, tag="ssum")
            nc.vector.reduce_sum(ssum[:], ET[:], axis=AX.X)
            rec = bigp.tile([C2, 1], F32, tag="rec")
            nc.vector.reciprocal(rec[:], ssum[:])
            ETn = bigp.tile([C2, C2], F32, tag="ETn")
            nc.vector.tensor_scalar_mul(ETn[:], ET[:], rec[:, 0:1])
            etp = ps.tile([128, 1024], F32, tag="dw", bufs=2)
            nc.tensor.transpose(etp[0:C2, 0:C2], ETn[:],
                                id_f32[0:C2, 0:C2])
            nc.scalar.copy(Ebd[:], etp[0:C2, 0:C2])

            # ---------- out2 = attn @ v, both batches per matmul ----------
            for w in range(NW):
                y0 = w * WR
                o2st = stgp.tile([C2, WR * W], FP16, tag="o2st", bufs=8)
                for pr in range(2):
                    pc0 = pr * 1024
                    o2ps = ps.tile([128, 1024], F32,
                                   tag=("cv" if pr == 0 else "dw"), bufs=2)
                    for hf in range(2):
                        nc.tensor.matmul(
                            o2ps[0:C2, hf * 512:(hf + 1) * 512], Ebd[:],
                            vl[0:C2, y0 * W + pc0 + hf * 512:
                               y0 * W + pc0 + (hf + 1) * 512],
                            start=True, stop=True)
                    if pr == 0:
                        nc.scalar.copy(o2st[:, pc0:pc0 + 1024],
                                       o2ps[0:C2, :])
                    else:
                        nc.vector.tensor_copy(o2st[:, pc0:pc0 + 1024],
                                              o2ps[0:C2, :])
                o2st3 = o2st.rearrange("p (r w) -> p r w", w=W)
                for b in range(BPC):
                    nc.gpsimd.dma_start(
                        out[b:b + 1, 96:128, y0:y0 + WR, :],
                        o2st3[C4 * b:C4 * b + C4, :, :])

        for _ in range(loops):
            one_pass()

    nc.compile()
    return nc


_NC_CACHE = None


def _get_nc():
    global _NC_CACHE
    if _NC_CACHE is None:
        _NC_CACHE = build_nc()
    return _NC_CACHE


def kernel(x, dw_w, dw_b, qkvl_w, qkvl_b):
    x = np.ascontiguousarray(np.asarray(x).astype(np.float16))
    shared = {
        "dw_w": np.ascontiguousarray(np.asarray(dw_w, dtype=np.float32)),
        "dw_b": np.ascontiguousarray(np.asarray(dw_b, dtype=np.float32)),
        "qkvl_w": np.ascontiguousarray(np.asarray(qkvl_w, dtype=np.float32)),
        "qkvl_b": np.ascontiguousarray(np.asarray(qkvl_b, dtype=np.float32)),
    }
    nc = _get_nc()
    in_maps = [
        {"x": x[c * BPC:(c + 1) * BPC], **shared} for c in range(N_CORES)
    ]
    res = bass_utils.run_bass_kernel_spmd(nc, in_maps,
                                          core_ids=list(range(N_CORES)))
    return np.concatenate(
        [np.asarray(res.results[c]["out"]).astype(np.float32)
         for c in range(N_CORES)], axis=0)
